# revision 9
# baseline (speedup 1.0000x reference)
"""Trainium2 Bass kernel for nn_Model_39676907883957 (dense_transformer).

Math (per batch element b, with S = D = N = 2048):
    q = Xq @ Wq^T + bq            # [S, D]
    kT = Wk @ Xk^T + bk[:, None]  # [D, S]  (k projected directly in transposed layout)
    v = Xv @ Wv^T + bv            # [S, D]
    scores[i, j] = sum_m q[m, i] * kT[m, j]          # q^T @ k^T
    attn = softmax_rows(scores)
    out[n, i] = sum_j v[j, n] * attn[i, j]           # == (attn @ v)^T

Sharding: data-parallel over batch, B=8 -> one batch element per NeuronCore.

Numerics: all matmuls on the PE in fp16 single-pass with fp32 PSUM
accumulation (measured rel err ~3.7e-3 vs the fp64 reference, against a
2e-2 gate). 5 x 2048^3 MACs per core.
"""

import numpy as np

import concourse.bass as bass
import concourse.bacc as bacc
import concourse.tile as tile
import concourse.mybir as mybir
from concourse.bass_utils import run_bass_kernel_spmd

B, S, D = 8, 2048, 2048
N = 2048                 # S == D
KT = N // 128            # 16 contraction tiles
NCHUNK = N // 512        # 4 free-dim chunks of 512
F16 = mybir.dt.float16
F32 = mybir.dt.float32
AX = mybir.AxisListType.X
EXP = mybir.ActivationFunctionType.Exp

_compiled = {}


def _build():
    nc = bacc.Bacc("TRN2", target_bir_lowering=False, debug=False)

    # ExternalInputs (per core). x* are host-transposed activations [d, s] fp16.
    xq = nc.dram_tensor("xq", [N, N], F16, kind="ExternalInput").ap()
    xk = nc.dram_tensor("xk", [N, N], F16, kind="ExternalInput").ap()
    xv = nc.dram_tensor("xv", [N, N], F16, kind="ExternalInput").ap()
    # host-transposed weights [d, e] fp16
    wq = nc.dram_tensor("wq", [N, N], F16, kind="ExternalInput").ap()
    wk = nc.dram_tensor("wk", [N, N], F16, kind="ExternalInput").ap()
    wv = nc.dram_tensor("wv", [N, N], F16, kind="ExternalInput").ap()
    # biases: bqb/bvb broadcast across partitions [128, N]; bkp partition-major [128, 16]
    bqb = nc.dram_tensor("bqb", [128, N], F32, kind="ExternalInput").ap()
    bkp = nc.dram_tensor("bkp", [128, KT], F32, kind="ExternalInput").ap()
    bvb = nc.dram_tensor("bvb", [128, N], F32, kind="ExternalInput").ap()

    out = nc.dram_tensor("out", [N, N], F16, kind="ExternalOutput").ap()

    with tile.TileContext(nc, pool_alloc_mode="queue") as tc:
        with tc.tile_pool(name="dram", bufs=1, space="DRAM") as dram:
            q_f = dram.tile([N, N], F16, tag="q_f")
            k_f = dram.tile([N, N], F16, tag="k_f")
            v_f = dram.tile([N, N], F16, tag="v_f")

            with tc.tile_pool(name="psum", bufs=8, space="PSUM") as psum:
                _proj_rows(nc, tc, psum, xq, wq, bqb, q_f, "q")
                _proj_cols(nc, tc, psum, xk, wk, bkp, k_f)
                _proj_rows(nc, tc, psum, xv, wv, bvb, v_f, "v")
                _attention(nc, tc, psum, q_f, k_f, v_f, out)

    nc.compile()
    return nc


def _load_kblock(nc, pool, dram_ap, col_blk, tag, dt=F16):
    """Load DRAM[:, col_blk*128 : +128] ([N, 128]) into one [128, N] SBUF tile
    whose slice [:, k*128:(k+1)*128] is contraction-tile k (partition = row%128)."""
    t = pool.tile([128, N], dt, tag=tag)
    src = dram_ap[:, col_blk * 128:(col_blk + 1) * 128].rearrange(
        "(t p) s -> p t s", p=128
    )
    dst = t[:].rearrange("p (t s) -> p t s", t=KT)
    nc.sync.dma_start(dst, src)
    return t


def _load_resident(nc, pool, src, tag):
    """Load an [N, N] fp16 DRAM tensor as KT resident [128, N] row-block tiles,
    chunk-0 columns first, on the SWDGE path (separate from HWDGE streaming)."""
    ts = [pool.tile([128, N], F16, tag=f"{tag}{k}", name=f"{tag}{k}") for k in range(KT)]
    for c in range(NCHUNK):
        cs = slice(c * 512, (c + 1) * 512)
        for k in range(KT):
            nc.gpsimd.dma_start(ts[k][:, cs], src[k * 128:(k + 1) * 128, cs])
    return ts


def _proj_rows(nc, tc, psum, x, w, bias_bcast, out_f, pfx):
    """Row-major projection: out[s, e] = sum_d X^T[d, s] * W^T[d, e] + bias[e].
    Stationary = activation k-blocks, moving = resident weights."""
    with (
        tc.tile_pool(name=f"p{pfx}_w", bufs=1) as wpool,
        tc.tile_pool(name=f"p{pfx}_x", bufs=2) as xpool,
        tc.tile_pool(name=f"p{pfx}_s", bufs=4) as spool,
        tc.tile_pool(name=f"p{pfx}_b", bufs=1) as bpool,
    ):
        bb = bpool.tile([128, N], F32, tag="bias")
        nc.sync.dma_start(bb[:], bias_bcast[:])
        w_t = _load_resident(nc, wpool, w, f"{pfx}w")
        for s in range(KT):
            a = _load_kblock(nc, xpool, x, s, f"{pfx}a")
            for c in range(NCHUNK):
                cs = slice(c * 512, (c + 1) * 512)
                ps = psum.tile([128, 512], F32)
                for k in range(KT):
                    nc.tensor.matmul(ps[:], a[:, k * 128:(k + 1) * 128],
                                     w_t[k][:, cs], start=(k == 0), stop=(k == KT - 1))
                o16 = spool.tile([128, 512], F16, tag="o16")
                nc.vector.tensor_add(o16[:], ps[:], bb[:, cs])
                nc.sync.dma_start(out_f[s * 128:(s + 1) * 128, cs], o16[:])


def _proj_cols(nc, tc, psum, x, w, bias_part, out_f):
    """kT-style projection: out[e, s] = sum_d W^T[d, e] * X^T[d, s] + bias[e].
    Stationary = weight k-blocks, moving = resident activations."""
    with (
        tc.tile_pool(name="pc_x", bufs=1) as xpool,
        tc.tile_pool(name="pc_w", bufs=2) as wpool,
        tc.tile_pool(name="pc_s", bufs=4) as spool,
        tc.tile_pool(name="pc_b", bufs=1) as bpool,
    ):
        bp = bpool.tile([128, KT], F32, tag="biasp")
        nc.sync.dma_start(bp[:], bias_part[:])
        x_t = _load_resident(nc, xpool, x, "kx")
        for e in range(KT):
            g = _load_kblock(nc, wpool, w, e, "kg")
            for c in range(NCHUNK):
                cs = slice(c * 512, (c + 1) * 512)
                ps = psum.tile([128, 512], F32)
                for k in range(KT):
                    nc.tensor.matmul(ps[:], g[:, k * 128:(k + 1) * 128],
                                     x_t[k][:, cs], start=(k == 0), stop=(k == KT - 1))
                o16 = spool.tile([128, 512], F16, tag="o16")
                nc.vector.tensor_scalar_add(o16[:], ps[:], bp[:, e:e + 1])
                nc.sync.dma_start(out_f[e * 128:(e + 1) * 128, cs], o16[:])


def _attention(nc, tc, psum, q_f, k_f, v_f, out):
    """Interleaved scores+softmax+attn@v, one 128-row attn block at a time.

    scores[i, j] = sum_m q[m, i]*kT[m, j]; row softmax -> a16 (row-tile i).
    a16 is block-transposed into 16 [128,128] attT tiles (j on partitions),
    then ov[i, n] = sum_j attn[i, j] * v[j, n] runs one iteration behind,
    consuming resident v row-blocks as the moving operand. out is written
    in [i, n] orientation; the host transposes. Per-row-tile transposes
    split across the sync and scalar HWDGE paths so neither queue stalls.
    """
    with (
        tc.tile_pool(name="sc_k", bufs=1) as kpool,
        tc.tile_pool(name="av_v", bufs=1) as vpool,
        tc.tile_pool(name="sc_a", bufs=3) as apool,
        tc.tile_pool(name="sc_q", bufs=3) as qpool,
        tc.tile_pool(name="sc_f", bufs=2) as fpool,
        tc.tile_pool(name="sc_s", bufs=3) as spool,
        tc.tile_pool(name="sc_t", bufs=4) as tpool,
        tc.tile_pool(name="av_s", bufs=4) as opool,
    ):
        k_t = _load_resident(nc, kpool, k_f, "sk")
        v_r = _load_resident(nc, vpool, v_f, "vr")

        def emit_av(attTJ, i):
            for c in range(NCHUNK):
                cs = slice(c * 512, (c + 1) * 512)
                ps = psum.tile([128, 512], F32)
                for j in range(KT):
                    nc.tensor.matmul(ps[:], attTJ[:, j * 128:(j + 1) * 128],
                                     v_r[j][:, cs], start=(j == 0), stop=(j == KT - 1))
                o16 = opool.tile([128, 512], F16, tag="o16")
                nc.vector.tensor_copy(o16[:], ps[:])
                nc.sync.dma_start(out[i * 128:(i + 1) * 128, cs], o16[:])

        pend = []
        for i in range(KT):
            qh = _load_kblock(nc, qpool, q_f, i, "sq")
            pss = []
            m4 = tpool.tile([128, NCHUNK], F32, tag="m4")
            for c in range(NCHUNK):
                cs = slice(c * 512, (c + 1) * 512)
                ps = psum.tile([128, 512], F32)
                for k in range(KT):
                    nc.tensor.matmul(ps[:], qh[:, k * 128:(k + 1) * 128],
                                     k_t[k][:, cs], start=(k == 0), stop=(k == KT - 1))
                nc.vector.reduce_max(m4[:, c:c + 1], ps[:], axis=AX)
                pss.append(ps)
            # attn@v runs two row-tiles behind so the scalar-engine
            # transposes have a full extra window of latency margin
            if len(pend) == 2:
                emit_av(*pend.pop(0))
            mx = tpool.tile([128, 1], F32, tag="mx")
            nc.vector.reduce_max(mx[:], m4[:], axis=AX)
            negm = tpool.tile([128, 1], F32, tag="negm")
            nc.scalar.mul(negm[:], mx[:], -1.0)
            af32 = fpool.tile([128, N], F32, tag="af32")
            sume = tpool.tile([128, NCHUNK], F32, tag="sume")
            for c in range(NCHUNK):
                cs = slice(c * 512, (c + 1) * 512)
                nc.scalar.activation(af32[:, cs], pss[c][:], EXP,
                                     bias=negm[:], scale=1.0,
                                     accum_out=sume[:, c:c + 1])
            tot = tpool.tile([128, 1], F32, tag="tot")
            nc.vector.reduce_sum(tot[:], sume[:], axis=AX)
            rcp = tpool.tile([128, 1], F32, tag="rcp")
            nc.vector.reciprocal(rcp[:], tot[:])
            a16 = spool.tile([128, N], F16, tag="a16")
            nc.vector.tensor_scalar_mul(a16[:], af32[:], rcp[:])
            attTJ = apool.tile([128, N], F16, tag="attTJ")
            for j in range(KT):
                nc.scalar.dma_start_transpose(
                    attTJ[:, j * 128:(j + 1) * 128],
                    a16[:, j * 128:(j + 1) * 128])
            pend.append((attTJ, i))
        for p in pend:
            emit_av(*p)


def prepare_in_maps(query, key_, value, Wq, bq, Wk, bk, Wv, bv):
    query = np.asarray(query, dtype=np.float32)
    key_ = np.asarray(key_, dtype=np.float32)
    value = np.asarray(value, dtype=np.float32)
    Wq = np.asarray(Wq, dtype=np.float32)
    Wk = np.asarray(Wk, dtype=np.float32)
    Wv = np.asarray(Wv, dtype=np.float32)
    bq = np.asarray(bq, dtype=np.float32)
    bk = np.asarray(bk, dtype=np.float32)
    bv = np.asarray(bv, dtype=np.float32)

    wqt = np.ascontiguousarray(Wq.T).astype(np.float16)
    wkt = np.ascontiguousarray(Wk.T).astype(np.float16)
    wvt = np.ascontiguousarray(Wv.T).astype(np.float16)
    bqb = np.broadcast_to(bq, (128, N)).copy()
    bvb = np.broadcast_to(bv, (128, N)).copy()
    bkp = np.ascontiguousarray(bk.reshape(KT, 128).T)

    in_maps = []
    for b in range(B):
        in_maps.append({
            "xq": np.ascontiguousarray(query[b].T).astype(np.float16),
            "xk": np.ascontiguousarray(key_[b].T).astype(np.float16),
            "xv": np.ascontiguousarray(value[b].T).astype(np.float16),
            "wq": wqt, "wk": wkt, "wv": wvt,
            "bqb": bqb, "bkp": bkp, "bvb": bvb,
        })
    return in_maps


def get_nc():
    if "nc" not in _compiled:
        _compiled["nc"] = _build()
    return _compiled["nc"]


def kernel(query, key_, value, Wq, bq, Wk, bk, Wv, bv):
    in_maps = prepare_in_maps(query, key_, value, Wq, bq, Wk, bk, Wv, bv)
    res = run_bass_kernel_spmd(get_nc(), in_maps, core_ids=list(range(B)))
    # device emits (attn @ v)[i, n]; the module's output is its transpose
    return np.stack([np.asarray(res.results[b]["out"]).T for b in range(B)]).astype(np.float32)


if __name__ == "__main__":
    rng = np.random.default_rng(0)
    inputs = {
        "query": rng.standard_normal((B, S, D), dtype=np.float32),
        "key_": rng.standard_normal((B, S, D), dtype=np.float32),
        "value": rng.standard_normal((B, S, D), dtype=np.float32),
        "Wq": (rng.standard_normal((D, D), dtype=np.float32) / np.sqrt(D)),
        "bq": rng.standard_normal(D).astype(np.float32) * 0.01,
        "Wk": (rng.standard_normal((D, D), dtype=np.float32) / np.sqrt(D)),
        "bk": rng.standard_normal(D).astype(np.float32) * 0.01,
        "Wv": (rng.standard_normal((D, D), dtype=np.float32) / np.sqrt(D)),
        "bv": rng.standard_normal(D).astype(np.float32) * 0.01,
    }
    out = kernel(**inputs)
    print("out", out.shape, out.dtype)


# revision 12
# speedup vs baseline: 1.0446x; 1.0446x over previous
"""Trainium2 Bass kernel for nn_Model_39676907883957 (dense_transformer).

Math (per batch element b, with S = D = N = 2048):
    q = Xq @ Wq^T + bq            # [S, D]
    kT = Wk @ Xk^T + bk[:, None]  # [D, S]  (k projected directly in transposed layout)
    v = Xv @ Wv^T + bv            # [S, D]
    scores[i, j] = sum_m q[m, i] * kT[m, j]          # q^T @ k^T
    attn = softmax_rows(scores)
    out[n, i] = sum_j v[j, n] * attn[i, j]           # == (attn @ v)^T

Sharding: data-parallel over batch, B=8 -> one batch element per NeuronCore.

Numerics: all matmuls on the PE in fp16 single-pass with fp32 PSUM
accumulation (measured rel err ~3.7e-3 vs the fp64 reference, against a
2e-2 gate). 5 x 2048^3 MACs per core.
"""

import numpy as np

import concourse.bass as bass
import concourse.bacc as bacc
import concourse.tile as tile
import concourse.mybir as mybir
from concourse.bass_utils import run_bass_kernel_spmd

B, S, D = 8, 2048, 2048
N = 2048                 # S == D
KT = N // 128            # 16 contraction tiles
NCHUNK = N // 512        # 4 free-dim chunks of 512
F16 = mybir.dt.float16
F32 = mybir.dt.float32
AX = mybir.AxisListType.X
EXP = mybir.ActivationFunctionType.Exp

_compiled = {}


def _build():
    nc = bacc.Bacc("TRN2", target_bir_lowering=False, debug=False)

    # ExternalInputs (per core). x* are host-transposed activations [d, s] fp16.
    xq = nc.dram_tensor("xq", [N, N], F16, kind="ExternalInput").ap()
    xk = nc.dram_tensor("xk", [N, N], F16, kind="ExternalInput").ap()
    xv = nc.dram_tensor("xv", [N, N], F16, kind="ExternalInput").ap()
    # host-transposed weights [d, e] fp16
    wq = nc.dram_tensor("wq", [N, N], F16, kind="ExternalInput").ap()
    wk = nc.dram_tensor("wk", [N, N], F16, kind="ExternalInput").ap()
    wv = nc.dram_tensor("wv", [N, N], F16, kind="ExternalInput").ap()
    # biases: bqb/bvb broadcast across partitions [128, N]; bkp partition-major [128, 16]
    bqb = nc.dram_tensor("bqb", [128, N], F32, kind="ExternalInput").ap()
    bkp = nc.dram_tensor("bkp", [128, KT], F32, kind="ExternalInput").ap()
    bvb = nc.dram_tensor("bvb", [128, N], F32, kind="ExternalInput").ap()

    out = nc.dram_tensor("out", [N, N], F16, kind="ExternalOutput").ap()

    with tile.TileContext(nc, pool_alloc_mode="queue") as tc:
        with tc.tile_pool(name="dram", bufs=1, space="DRAM") as dram:
            q_f = dram.tile([N, N], F16, tag="q_f")
            k_f = dram.tile([N, N], F16, tag="k_f")
            v_f = dram.tile([N, N], F16, tag="v_f")
            at_f = dram.tile([N, N], F16, tag="at_f")

            with tc.tile_pool(name="psum", bufs=8, space="PSUM") as psum:
                _proj_rows(nc, tc, psum, xq, wq, bqb, q_f, "q")
                _proj_cols(nc, tc, psum, xk, wk, bkp, k_f)
                _proj_rows(nc, tc, psum, xv, wv, bvb, v_f, "v")
                _attention(nc, tc, psum, q_f, k_f, v_f, at_f, out)

    nc.compile()
    return nc


def _load_kblock(nc, pool, dram_ap, col_blk, tag, dt=F16):
    """Load DRAM[:, col_blk*128 : +128] ([N, 128]) into one [128, N] SBUF tile
    whose slice [:, k*128:(k+1)*128] is contraction-tile k (partition = row%128)."""
    t = pool.tile([128, N], dt, tag=tag)
    src = dram_ap[:, col_blk * 128:(col_blk + 1) * 128].rearrange(
        "(t p) s -> p t s", p=128
    )
    dst = t[:].rearrange("p (t s) -> p t s", t=KT)
    nc.sync.dma_start(dst, src)
    return t


def _load_resident(nc, pool, src, tag):
    """Load an [N, N] fp16 DRAM tensor as KT resident [128, N] row-block tiles,
    chunk-0 columns first, on the SWDGE path (separate from HWDGE streaming)."""
    ts = [pool.tile([128, N], F16, tag=f"{tag}{k}", name=f"{tag}{k}") for k in range(KT)]
    for c in range(NCHUNK):
        cs = slice(c * 512, (c + 1) * 512)
        for k in range(KT):
            nc.gpsimd.dma_start(ts[k][:, cs], src[k * 128:(k + 1) * 128, cs])
    return ts


def _proj_rows(nc, tc, psum, x, w, bias_bcast, out_f, pfx):
    """Row-major projection: out[s, e] = sum_d X^T[d, s] * W^T[d, e] + bias[e].
    Stationary = activation k-blocks, moving = resident weights."""
    with (
        tc.tile_pool(name=f"p{pfx}_w", bufs=1) as wpool,
        tc.tile_pool(name=f"p{pfx}_x", bufs=2) as xpool,
        tc.tile_pool(name=f"p{pfx}_s", bufs=4) as spool,
        tc.tile_pool(name=f"p{pfx}_b", bufs=1) as bpool,
    ):
        bb = bpool.tile([128, N], F32, tag="bias")
        nc.sync.dma_start(bb[:], bias_bcast[:])
        w_t = _load_resident(nc, wpool, w, f"{pfx}w")
        for s in range(KT):
            a = _load_kblock(nc, xpool, x, s, f"{pfx}a")
            for c in range(NCHUNK):
                cs = slice(c * 512, (c + 1) * 512)
                ps = psum.tile([128, 512], F32)
                for k in range(KT):
                    nc.tensor.matmul(ps[:], a[:, k * 128:(k + 1) * 128],
                                     w_t[k][:, cs], start=(k == 0), stop=(k == KT - 1))
                o16 = spool.tile([128, 512], F16, tag="o16")
                nc.vector.tensor_add(o16[:], ps[:], bb[:, cs])
                nc.sync.dma_start(out_f[s * 128:(s + 1) * 128, cs], o16[:])


def _proj_cols(nc, tc, psum, x, w, bias_part, out_f):
    """kT-style projection: out[e, s] = sum_d W^T[d, e] * X^T[d, s] + bias[e].
    Stationary = weight k-blocks, moving = resident activations."""
    with (
        tc.tile_pool(name="pc_x", bufs=1) as xpool,
        tc.tile_pool(name="pc_w", bufs=2) as wpool,
        tc.tile_pool(name="pc_s", bufs=4) as spool,
        tc.tile_pool(name="pc_b", bufs=1) as bpool,
    ):
        bp = bpool.tile([128, KT], F32, tag="biasp")
        nc.sync.dma_start(bp[:], bias_part[:])
        x_t = _load_resident(nc, xpool, x, "kx")
        for e in range(KT):
            g = _load_kblock(nc, wpool, w, e, "kg")
            for c in range(NCHUNK):
                cs = slice(c * 512, (c + 1) * 512)
                ps = psum.tile([128, 512], F32)
                for k in range(KT):
                    nc.tensor.matmul(ps[:], g[:, k * 128:(k + 1) * 128],
                                     x_t[k][:, cs], start=(k == 0), stop=(k == KT - 1))
                o16 = spool.tile([128, 512], F16, tag="o16")
                nc.vector.tensor_scalar_add(o16[:], ps[:], bp[:, e:e + 1])
                nc.sync.dma_start(out_f[e * 128:(e + 1) * 128, cs], o16[:])


LAG = 5                   # row-tiles attn@v trails behind scores
GRP = 4                   # row-tiles per transpose group


def _attention(nc, tc, psum, q_f, k_f, v_f, at_f, out):
    """Interleaved scores+softmax+attn@v, one 128-row attn block at a time.

    scores[i, j] = sum_m q[m, i]*kT[m, j]; row softmax -> a16 -> at_f DRAM.
    Every GRP row-tiles, 16 [GRP*128, 128] DRAM->SBUF transposes (scalar
    HWDGE) build attn^T group tiles (j on partitions). attn@v for row-tile
    i runs LAG row-tiles behind: ov[i, n] = sum_j attn[i, j] * v[j, n],
    with resident v row-blocks as the moving operand. out is written in
    [i, n] orientation; the host transposes.
    """
    with (
        tc.tile_pool(name="sc_k", bufs=1) as kpool,
        tc.tile_pool(name="av_v", bufs=1) as vpool,
        tc.tile_pool(name="sc_a", bufs=2) as apool,
        tc.tile_pool(name="sc_q", bufs=3) as qpool,
        tc.tile_pool(name="sc_f", bufs=1) as fpool,
        tc.tile_pool(name="sc_s", bufs=2) as spool,
        tc.tile_pool(name="sc_t", bufs=4) as tpool,
        tc.tile_pool(name="av_s", bufs=4) as opool,
    ):
        k_t = _load_resident(nc, kpool, k_f, "sk")
        v_r = _load_resident(nc, vpool, v_f, "vr")
        attg_of = {}

        def emit_av(i):
            attg = attg_of[i // GRP]
            ts = slice((i % GRP) * 128, (i % GRP) * 128 + 128)
            for c in range(NCHUNK):
                cs = slice(c * 512, (c + 1) * 512)
                ps = psum.tile([128, 512], F32)
                for j in range(KT):
                    nc.tensor.matmul(ps[:], attg[j][:, ts],
                                     v_r[j][:, cs], start=(j == 0), stop=(j == KT - 1))
                o16 = opool.tile([128, 512], F16, tag="o16")
                nc.vector.tensor_copy(o16[:], ps[:])
                nc.sync.dma_start(out[i * 128:(i + 1) * 128, cs], o16[:])

        for i in range(KT):
            qh = _load_kblock(nc, qpool, q_f, i, "sq")
            pss = []
            m4 = tpool.tile([128, NCHUNK], F32, tag="m4")
            for c in range(NCHUNK):
                cs = slice(c * 512, (c + 1) * 512)
                ps = psum.tile([128, 512], F32)
                for k in range(KT):
                    nc.tensor.matmul(ps[:], qh[:, k * 128:(k + 1) * 128],
                                     k_t[k][:, cs], start=(k == 0), stop=(k == KT - 1))
                nc.vector.reduce_max(m4[:, c:c + 1], ps[:], axis=AX)
                pss.append(ps)
            mx = tpool.tile([128, 1], F32, tag="mx")
            nc.vector.reduce_max(mx[:], m4[:], axis=AX)
            negm = tpool.tile([128, 1], F32, tag="negm")
            nc.scalar.mul(negm[:], mx[:], -1.0)
            af32 = fpool.tile([128, N], F32, tag="af32")
            sume = tpool.tile([128, NCHUNK], F32, tag="sume")
            for c in range(NCHUNK):
                cs = slice(c * 512, (c + 1) * 512)
                nc.scalar.activation(af32[:, cs], pss[c][:], EXP,
                                     bias=negm[:], scale=1.0,
                                     accum_out=sume[:, c:c + 1])
            tot = tpool.tile([128, 1], F32, tag="tot")
            nc.vector.reduce_sum(tot[:], sume[:], axis=AX)
            rcp = tpool.tile([128, 1], F32, tag="rcp")
            nc.vector.reciprocal(rcp[:], tot[:])
            a16 = spool.tile([128, N], F16, tag="a16")
            nc.vector.tensor_scalar_mul(a16[:], af32[:], rcp[:])
            nc.sync.dma_start(at_f[i * 128:(i + 1) * 128, :], a16[:])
            if i % GRP == GRP - 1:
                g = i // GRP
                attg = [apool.tile([128, GRP * 128], F16, tag=f"ag{j}",
                                   name=f"ag{j}") for j in range(KT)]
                for j in range(KT):
                    nc.scalar.dma_start_transpose(
                        attg[j][:],
                        at_f[g * GRP * 128:(g + 1) * GRP * 128,
                             j * 128:(j + 1) * 128])
                attg_of[g] = attg
            if i >= LAG:
                emit_av(i - LAG)
        for i in range(KT - LAG, KT):
            emit_av(i)


def prepare_in_maps(query, key_, value, Wq, bq, Wk, bk, Wv, bv):
    query = np.asarray(query, dtype=np.float32)
    key_ = np.asarray(key_, dtype=np.float32)
    value = np.asarray(value, dtype=np.float32)
    Wq = np.asarray(Wq, dtype=np.float32)
    Wk = np.asarray(Wk, dtype=np.float32)
    Wv = np.asarray(Wv, dtype=np.float32)
    bq = np.asarray(bq, dtype=np.float32)
    bk = np.asarray(bk, dtype=np.float32)
    bv = np.asarray(bv, dtype=np.float32)

    wqt = np.ascontiguousarray(Wq.T).astype(np.float16)
    wkt = np.ascontiguousarray(Wk.T).astype(np.float16)
    wvt = np.ascontiguousarray(Wv.T).astype(np.float16)
    bqb = np.broadcast_to(bq, (128, N)).copy()
    bvb = np.broadcast_to(bv, (128, N)).copy()
    bkp = np.ascontiguousarray(bk.reshape(KT, 128).T)

    in_maps = []
    for b in range(B):
        in_maps.append({
            "xq": np.ascontiguousarray(query[b].T).astype(np.float16),
            "xk": np.ascontiguousarray(key_[b].T).astype(np.float16),
            "xv": np.ascontiguousarray(value[b].T).astype(np.float16),
            "wq": wqt, "wk": wkt, "wv": wvt,
            "bqb": bqb, "bkp": bkp, "bvb": bvb,
        })
    return in_maps


def get_nc():
    if "nc" not in _compiled:
        _compiled["nc"] = _build()
    return _compiled["nc"]


def kernel(query, key_, value, Wq, bq, Wk, bk, Wv, bv):
    in_maps = prepare_in_maps(query, key_, value, Wq, bq, Wk, bk, Wv, bv)
    res = run_bass_kernel_spmd(get_nc(), in_maps, core_ids=list(range(B)))
    # device emits (attn @ v)[i, n]; the module's output is its transpose
    return np.stack([np.asarray(res.results[b]["out"]).T for b in range(B)]).astype(np.float32)


if __name__ == "__main__":
    rng = np.random.default_rng(0)
    inputs = {
        "query": rng.standard_normal((B, S, D), dtype=np.float32),
        "key_": rng.standard_normal((B, S, D), dtype=np.float32),
        "value": rng.standard_normal((B, S, D), dtype=np.float32),
        "Wq": (rng.standard_normal((D, D), dtype=np.float32) / np.sqrt(D)),
        "bq": rng.standard_normal(D).astype(np.float32) * 0.01,
        "Wk": (rng.standard_normal((D, D), dtype=np.float32) / np.sqrt(D)),
        "bk": rng.standard_normal(D).astype(np.float32) * 0.01,
        "Wv": (rng.standard_normal((D, D), dtype=np.float32) / np.sqrt(D)),
        "bv": rng.standard_normal(D).astype(np.float32) * 0.01,
    }
    out = kernel(**inputs)
    print("out", out.shape, out.dtype)


# revision 19
# speedup vs baseline: 1.0578x; 1.0126x over previous
"""Trainium2 Bass kernel for nn_Model_39676907883957 (dense_transformer).

Math (per batch element b, with S = D = N = 2048):
    q = Xq @ Wq^T + bq            # [S, D]
    kT = Wk @ Xk^T + bk[:, None]  # [D, S]  (k projected directly in transposed layout)
    v = Xv @ Wv^T + bv            # [S, D]
    scores[i, j] = sum_m q[m, i] * kT[m, j]          # q^T @ k^T
    attn = softmax_rows(scores)
    out[n, i] = sum_j v[j, n] * attn[i, j]           # == (attn @ v)^T

Sharding: data-parallel over batch, B=8 -> one batch element per NeuronCore.

Numerics: all matmuls on the PE in fp16 single-pass with fp32 PSUM
accumulation (measured rel err ~3.7e-3 vs the fp64 reference, against a
2e-2 gate). 5 x 2048^3 MACs per core.
"""

import numpy as np

import concourse.bass as bass
import concourse.bacc as bacc
import concourse.tile as tile
import concourse.mybir as mybir
from concourse.bass_utils import run_bass_kernel_spmd

B, S, D = 8, 2048, 2048
N = 2048                 # S == D
KT = N // 128            # 16 contraction tiles
NCHUNK = N // 512        # 4 free-dim chunks of 512
F16 = mybir.dt.float16
F32 = mybir.dt.float32
AX = mybir.AxisListType.X
EXP = mybir.ActivationFunctionType.Exp

_compiled = {}


def _build():
    nc = bacc.Bacc("TRN2", target_bir_lowering=False, debug=False)

    # ExternalInputs (per core). x* are host-transposed activations [d, s] fp16.
    xq = nc.dram_tensor("xq", [N, N], F16, kind="ExternalInput").ap()
    xk = nc.dram_tensor("xk", [N, N], F16, kind="ExternalInput").ap()
    xv = nc.dram_tensor("xv", [N, N], F16, kind="ExternalInput").ap()
    # host-transposed weights [d, e] fp16
    wq = nc.dram_tensor("wq", [N, N], F16, kind="ExternalInput").ap()
    wk = nc.dram_tensor("wk", [N, N], F16, kind="ExternalInput").ap()
    wv = nc.dram_tensor("wv", [N, N], F16, kind="ExternalInput").ap()
    # biases: bqb/bvb broadcast across partitions [128, N]; bkp partition-major [128, 16]
    bqb = nc.dram_tensor("bqb", [128, N], F32, kind="ExternalInput").ap()
    bkp = nc.dram_tensor("bkp", [128, KT], F32, kind="ExternalInput").ap()
    bvb = nc.dram_tensor("bvb", [128, N], F32, kind="ExternalInput").ap()

    out = nc.dram_tensor("out", [N, N], F16, kind="ExternalOutput").ap()

    with tile.TileContext(nc, pool_alloc_mode="queue") as tc:
        with tc.tile_pool(name="dram", bufs=1, space="DRAM") as dram:
            q_f = dram.tile([N, N], F16, tag="q_f")
            k_f = dram.tile([N, N], F16, tag="k_f")
            v_f = dram.tile([N, N], F16, tag="v_f")
            at_f = dram.tile([N, N], F16, tag="at_f")

            with tc.tile_pool(name="psum", bufs=8, space="PSUM") as psum:
                _proj_rows(nc, tc, psum, xq, wq, bqb, q_f, "q")
                _proj_cols(nc, tc, psum, xk, wk, bkp, k_f)
                _proj_rows(nc, tc, psum, xv, wv, bvb, v_f, "v")
                _attention(nc, tc, psum, q_f, k_f, v_f, at_f, out)

    nc.compile()
    return nc


def _load_kblock(nc, pool, dram_ap, col_blk, tag, dt=F16):
    """Load DRAM[:, col_blk*128 : +128] ([N, 128]) into one [128, N] SBUF tile
    whose slice [:, k*128:(k+1)*128] is contraction-tile k (partition = row%128)."""
    t = pool.tile([128, N], dt, tag=tag)
    src = dram_ap[:, col_blk * 128:(col_blk + 1) * 128].rearrange(
        "(t p) s -> p t s", p=128
    )
    dst = t[:].rearrange("p (t s) -> p t s", t=KT)
    nc.sync.dma_start(dst, src)
    return t


def _load_resident(nc, pool, src, tag, split0=False):
    """Load an [N, N] fp16 DRAM tensor as KT resident [128, N] row-block tiles,
    chunk-0 columns first, on the SWDGE path (separate from HWDGE streaming).
    split0 puts half of chunk 0 on the sync path to halve cold-start latency."""
    ts = [pool.tile([128, N], F16, tag=f"{tag}{k}", name=f"{tag}{k}") for k in range(KT)]
    for c in range(NCHUNK):
        cs = slice(c * 512, (c + 1) * 512)
        for k in range(KT):
            eng = nc.sync if (split0 and c == 0 and k % 2 == 1) else nc.gpsimd
            eng.dma_start(ts[k][:, cs], src[k * 128:(k + 1) * 128, cs])
    return ts


def _proj_rows(nc, tc, psum, x, w, bias_bcast, out_f, pfx):
    """Row-major projection: out[s, e] = sum_d X^T[d, s] * W^T[d, e] + bias[e].
    Stationary = activation k-blocks, moving = resident weights."""
    with (
        tc.tile_pool(name=f"p{pfx}_w", bufs=1) as wpool,
        tc.tile_pool(name=f"p{pfx}_x", bufs=3) as xpool,
        tc.tile_pool(name=f"p{pfx}_s", bufs=4) as spool,
        tc.tile_pool(name=f"p{pfx}_b", bufs=1) as bpool,
    ):
        pend = {0: _load_kblock(nc, xpool, x, 0, f"{pfx}a")}
        bb = bpool.tile([128, N], F32, tag="bias")
        nc.sync.dma_start(bb[:], bias_bcast[:])
        w_t = _load_resident(nc, wpool, w, f"{pfx}w", split0=True)
        for s in range(KT):
            if s + 1 < KT:
                pend[s + 1] = _load_kblock(nc, xpool, x, s + 1, f"{pfx}a")
            a = pend.pop(s)
            for c in range(NCHUNK):
                cs = slice(c * 512, (c + 1) * 512)
                ps = psum.tile([128, 512], F32)
                for k in range(KT):
                    nc.tensor.matmul(ps[:], a[:, k * 128:(k + 1) * 128],
                                     w_t[k][:, cs], start=(k == 0), stop=(k == KT - 1))
                o16 = spool.tile([128, 512], F16, tag="o16")
                nc.vector.tensor_add(o16[:], ps[:], bb[:, cs])
                nc.sync.dma_start(out_f[s * 128:(s + 1) * 128, cs], o16[:])


def _proj_cols(nc, tc, psum, x, w, bias_part, out_f):
    """kT-style projection: out[e, s] = sum_d W^T[d, e] * X^T[d, s] + bias[e].
    Stationary = weight k-blocks, moving = resident activations."""
    with (
        tc.tile_pool(name="pc_x", bufs=1) as xpool,
        tc.tile_pool(name="pc_w", bufs=3) as wpool,
        tc.tile_pool(name="pc_s", bufs=4) as spool,
        tc.tile_pool(name="pc_b", bufs=1) as bpool,
    ):
        pend = {0: _load_kblock(nc, wpool, w, 0, "kg")}
        bp = bpool.tile([128, KT], F32, tag="biasp")
        nc.sync.dma_start(bp[:], bias_part[:])
        x_t = _load_resident(nc, xpool, x, "kx")
        for e in range(KT):
            if e + 1 < KT:
                pend[e + 1] = _load_kblock(nc, wpool, w, e + 1, "kg")
            g = pend.pop(e)
            for c in range(NCHUNK):
                cs = slice(c * 512, (c + 1) * 512)
                ps = psum.tile([128, 512], F32)
                for k in range(KT):
                    nc.tensor.matmul(ps[:], g[:, k * 128:(k + 1) * 128],
                                     x_t[k][:, cs], start=(k == 0), stop=(k == KT - 1))
                o16 = spool.tile([128, 512], F16, tag="o16")
                nc.vector.tensor_scalar_add(o16[:], ps[:], bp[:, e:e + 1])
                nc.sync.dma_start(out_f[e * 128:(e + 1) * 128, cs], o16[:])


LAG = 5                   # row-tiles attn@v trails behind scores
GRP = 4                   # row-tiles per transpose group


def _attention(nc, tc, psum, q_f, k_f, v_f, at_f, out):
    """Interleaved scores+softmax+attn@v, one 128-row attn block at a time.

    scores[i, j] = sum_m q[m, i]*kT[m, j]; row softmax -> a16 -> at_f DRAM.
    Every GRP row-tiles, 16 [GRP*128, 128] DRAM->SBUF transposes (scalar
    HWDGE) build attn^T group tiles (j on partitions). attn@v for row-tile
    i runs LAG row-tiles behind: ov[i, n] = sum_j attn[i, j] * v[j, n],
    with resident v row-blocks as the moving operand. out is written in
    [i, n] orientation; the host transposes.
    """
    with (
        tc.tile_pool(name="sc_k", bufs=1) as kpool,
        tc.tile_pool(name="av_v", bufs=1) as vpool,
        tc.tile_pool(name="sc_a", bufs=2) as apool,
        tc.tile_pool(name="sc_q", bufs=3) as qpool,
        tc.tile_pool(name="sc_c", bufs=2) as cpool,
        tc.tile_pool(name="sc_s", bufs=2) as spool,
        tc.tile_pool(name="sc_t", bufs=4) as tpool,
        tc.tile_pool(name="sc_r", bufs=8) as rpool,
        tc.tile_pool(name="av_s", bufs=4) as opool,
    ):
        k_t = _load_resident(nc, kpool, k_f, "sk")
        v_r = _load_resident(nc, vpool, v_f, "vr")
        attg_of = {}
        rcp_of = {}

        def emit_av(i):
            attg = attg_of[i // GRP]
            ts = slice((i % GRP) * 128, (i % GRP) * 128 + 128)
            rcp = rcp_of.pop(i)
            for c in range(NCHUNK):
                cs = slice(c * 512, (c + 1) * 512)
                ps = psum.tile([128, 512], F32)
                for j in range(KT):
                    nc.tensor.matmul(ps[:], attg[j][:, ts],
                                     v_r[j][:, cs], start=(j == 0), stop=(j == KT - 1))
                # attn rows are stored unnormalized; fold the softmax
                # 1/rowsum into the output copy (per-partition scalar)
                o16 = opool.tile([128, 512], F16, tag="o16")
                nc.vector.tensor_scalar_mul(o16[:], ps[:], rcp[:])
                nc.sync.dma_start(out[i * 128:(i + 1) * 128, cs], o16[:])

        qpend = {0: _load_kblock(nc, qpool, q_f, 0, "sq"),
                 1: _load_kblock(nc, qpool, q_f, 1, "sq")}
        for i in range(KT):
            if i + 2 < KT:
                qpend[i + 2] = _load_kblock(nc, qpool, q_f, i + 2, "sq")
            qh = qpend.pop(i)
            m4 = tpool.tile([128, NCHUNK], F32, tag="m4")
            sc32 = cpool.tile([128, N], F32, tag="sc32")
            for c in range(NCHUNK):
                cs = slice(c * 512, (c + 1) * 512)
                ps = psum.tile([128, 512], F32)
                for k in range(KT):
                    nc.tensor.matmul(ps[:], qh[:, k * 128:(k + 1) * 128],
                                     k_t[k][:, cs], start=(k == 0), stop=(k == KT - 1))
                nc.vector.reduce_max(m4[:, c:c + 1], ps[:], axis=AX)
                # drain the PSUM bank immediately so scores never starve
                # on banks even when the scalar engine runs behind
                nc.vector.tensor_copy(sc32[:, cs], ps[:])
            mx = tpool.tile([128, 1], F32, tag="mx")
            nc.vector.reduce_max(mx[:], m4[:], axis=AX)
            negm = tpool.tile([128, 1], F32, tag="negm")
            nc.scalar.mul(negm[:], mx[:], -1.0)
            a16 = spool.tile([128, N], F16, tag="a16")
            sume = tpool.tile([128, NCHUNK], F32, tag="sume")
            for c in range(NCHUNK):
                cs = slice(c * 512, (c + 1) * 512)
                nc.scalar.activation(a16[:, cs], sc32[:, cs], EXP,
                                     bias=negm[:], scale=1.0,
                                     accum_out=sume[:, c:c + 1])
            tot = tpool.tile([128, 1], F32, tag="tot")
            nc.vector.reduce_sum(tot[:], sume[:], axis=AX)
            rcp = rpool.tile([128, 1], F32, tag="rcp")
            nc.vector.reciprocal(rcp[:], tot[:])
            rcp_of[i] = rcp
            nc.sync.dma_start(at_f[i * 128:(i + 1) * 128, :], a16[:])
            if i % GRP == GRP - 1:
                g = i // GRP
                attg = [apool.tile([128, GRP * 128], F16, tag=f"ag{j}",
                                   name=f"ag{j}") for j in range(KT)]
                for j in range(KT):
                    nc.scalar.dma_start_transpose(
                        attg[j][:],
                        at_f[g * GRP * 128:(g + 1) * GRP * 128,
                             j * 128:(j + 1) * 128])
                attg_of[g] = attg
            if i >= LAG:
                emit_av(i - LAG)
        for i in range(KT - LAG, KT):
            emit_av(i)


def prepare_in_maps(query, key_, value, Wq, bq, Wk, bk, Wv, bv):
    query = np.asarray(query, dtype=np.float32)
    key_ = np.asarray(key_, dtype=np.float32)
    value = np.asarray(value, dtype=np.float32)
    Wq = np.asarray(Wq, dtype=np.float32)
    Wk = np.asarray(Wk, dtype=np.float32)
    Wv = np.asarray(Wv, dtype=np.float32)
    bq = np.asarray(bq, dtype=np.float32)
    bk = np.asarray(bk, dtype=np.float32)
    bv = np.asarray(bv, dtype=np.float32)

    wqt = np.ascontiguousarray(Wq.T).astype(np.float16)
    wkt = np.ascontiguousarray(Wk.T).astype(np.float16)
    wvt = np.ascontiguousarray(Wv.T).astype(np.float16)
    bqb = np.broadcast_to(bq, (128, N)).copy()
    bvb = np.broadcast_to(bv, (128, N)).copy()
    bkp = np.ascontiguousarray(bk.reshape(KT, 128).T)

    in_maps = []
    for b in range(B):
        in_maps.append({
            "xq": np.ascontiguousarray(query[b].T).astype(np.float16),
            "xk": np.ascontiguousarray(key_[b].T).astype(np.float16),
            "xv": np.ascontiguousarray(value[b].T).astype(np.float16),
            "wq": wqt, "wk": wkt, "wv": wvt,
            "bqb": bqb, "bkp": bkp, "bvb": bvb,
        })
    return in_maps


def get_nc():
    if "nc" not in _compiled:
        _compiled["nc"] = _build()
    return _compiled["nc"]


def kernel(query, key_, value, Wq, bq, Wk, bk, Wv, bv):
    in_maps = prepare_in_maps(query, key_, value, Wq, bq, Wk, bk, Wv, bv)
    res = run_bass_kernel_spmd(get_nc(), in_maps, core_ids=list(range(B)))
    # device emits (attn @ v)[i, n]; the module's output is its transpose
    return np.stack([np.asarray(res.results[b]["out"]).T for b in range(B)]).astype(np.float32)


if __name__ == "__main__":
    rng = np.random.default_rng(0)
    inputs = {
        "query": rng.standard_normal((B, S, D), dtype=np.float32),
        "key_": rng.standard_normal((B, S, D), dtype=np.float32),
        "value": rng.standard_normal((B, S, D), dtype=np.float32),
        "Wq": (rng.standard_normal((D, D), dtype=np.float32) / np.sqrt(D)),
        "bq": rng.standard_normal(D).astype(np.float32) * 0.01,
        "Wk": (rng.standard_normal((D, D), dtype=np.float32) / np.sqrt(D)),
        "bk": rng.standard_normal(D).astype(np.float32) * 0.01,
        "Wv": (rng.standard_normal((D, D), dtype=np.float32) / np.sqrt(D)),
        "bv": rng.standard_normal(D).astype(np.float32) * 0.01,
    }
    out = kernel(**inputs)
    print("out", out.shape, out.dtype)


# revision 20
# speedup vs baseline: 1.0760x; 1.0173x over previous
"""Trainium2 Bass kernel for nn_Model_39676907883957 (dense_transformer).

Math (per batch element b, with S = D = N = 2048):
    q = Xq @ Wq^T + bq            # [S, D]
    kT = Wk @ Xk^T + bk[:, None]  # [D, S]  (k projected directly in transposed layout)
    v = Xv @ Wv^T + bv            # [S, D]
    scores[i, j] = sum_m q[m, i] * kT[m, j]          # q^T @ k^T
    attn = softmax_rows(scores)
    out[n, i] = sum_j v[j, n] * attn[i, j]           # == (attn @ v)^T

Sharding: data-parallel over batch, B=8 -> one batch element per NeuronCore.

Numerics: all matmuls on the PE in fp16 single-pass with fp32 PSUM
accumulation (measured rel err ~3.7e-3 vs the fp64 reference, against a
2e-2 gate). 5 x 2048^3 MACs per core.
"""

import numpy as np

import concourse.bass as bass
import concourse.bacc as bacc
import concourse.tile as tile
import concourse.mybir as mybir
from concourse.bass_utils import run_bass_kernel_spmd

B, S, D = 8, 2048, 2048
N = 2048                 # S == D
KT = N // 128            # 16 contraction tiles
NCHUNK = N // 512        # 4 free-dim chunks of 512
F16 = mybir.dt.float16
F32 = mybir.dt.float32
AX = mybir.AxisListType.X
EXP = mybir.ActivationFunctionType.Exp

_compiled = {}


def _build():
    nc = bacc.Bacc("TRN2", target_bir_lowering=False, debug=False)

    # ExternalInputs (per core). x* are host-transposed activations [d, s] fp16.
    xq = nc.dram_tensor("xq", [N, N], F16, kind="ExternalInput").ap()
    xk = nc.dram_tensor("xk", [N, N], F16, kind="ExternalInput").ap()
    xv = nc.dram_tensor("xv", [N, N], F16, kind="ExternalInput").ap()
    # host-transposed weights [d, e] fp16
    wq = nc.dram_tensor("wq", [N, N], F16, kind="ExternalInput").ap()
    wk = nc.dram_tensor("wk", [N, N], F16, kind="ExternalInput").ap()
    wv = nc.dram_tensor("wv", [N, N], F16, kind="ExternalInput").ap()
    # biases: bqb/bvb broadcast across partitions [128, N]; bkp partition-major [128, 16]
    bqb = nc.dram_tensor("bqb", [128, N], F32, kind="ExternalInput").ap()
    bkp = nc.dram_tensor("bkp", [128, KT], F32, kind="ExternalInput").ap()
    bvb = nc.dram_tensor("bvb", [128, N], F32, kind="ExternalInput").ap()

    out = nc.dram_tensor("out", [N, N], F16, kind="ExternalOutput").ap()

    with tile.TileContext(nc, pool_alloc_mode="queue") as tc:
        with tc.tile_pool(name="dram", bufs=1, space="DRAM") as dram:
            q_f = dram.tile([N, N], F16, tag="q_f")
            k_f = dram.tile([N, N], F16, tag="k_f")
            v_f = dram.tile([N, N], F16, tag="v_f")
            at_f = dram.tile([N, N], F16, tag="at_f")

            with tc.tile_pool(name="psum", bufs=8, space="PSUM") as psum:
                _proj_rows(nc, tc, psum, xq, wq, bqb, q_f, "q")
                _proj_cols(nc, tc, psum, xk, wk, bkp, k_f)
                _proj_rows(nc, tc, psum, xv, wv, bvb, v_f, "v")
                _attention(nc, tc, psum, q_f, k_f, v_f, at_f, out)

    nc.compile()
    return nc


def _load_kblock(nc, pool, dram_ap, col_blk, tag, dt=F16):
    """Load DRAM[:, col_blk*128 : +128] ([N, 128]) into one [128, N] SBUF tile
    whose slice [:, k*128:(k+1)*128] is contraction-tile k (partition = row%128)."""
    t = pool.tile([128, N], dt, tag=tag)
    src = dram_ap[:, col_blk * 128:(col_blk + 1) * 128].rearrange(
        "(t p) s -> p t s", p=128
    )
    dst = t[:].rearrange("p (t s) -> p t s", t=KT)
    nc.sync.dma_start(dst, src)
    return t


def _load_resident(nc, pool, src, tag, split0=False):
    """Load an [N, N] fp16 DRAM tensor as KT resident [128, N] row-block tiles,
    chunk-0 columns first, on the SWDGE path (separate from HWDGE streaming).
    split0 puts half of chunk 0 on the sync path to halve cold-start latency."""
    ts = [pool.tile([128, N], F16, tag=f"{tag}{k}", name=f"{tag}{k}") for k in range(KT)]
    for c in range(NCHUNK):
        cs = slice(c * 512, (c + 1) * 512)
        for k in range(KT):
            eng = nc.sync if (split0 and c == 0 and k % 2 == 1) else nc.gpsimd
            eng.dma_start(ts[k][:, cs], src[k * 128:(k + 1) * 128, cs])
    return ts


def _proj_rows(nc, tc, psum, x, w, bias_bcast, out_f, pfx):
    """Row-major projection: out[s, e] = sum_d X^T[d, s] * W^T[d, e] + bias[e].
    Stationary = activation k-blocks, moving = resident weights."""
    with (
        tc.tile_pool(name=f"p{pfx}_w", bufs=1) as wpool,
        tc.tile_pool(name=f"p{pfx}_x", bufs=3) as xpool,
        tc.tile_pool(name=f"p{pfx}_s", bufs=4) as spool,
        tc.tile_pool(name=f"p{pfx}_b", bufs=1) as bpool,
    ):
        pend = {0: _load_kblock(nc, xpool, x, 0, f"{pfx}a")}
        bb = bpool.tile([128, N], F32, tag="bias")
        nc.sync.dma_start(bb[:], bias_bcast[:])
        w_t = _load_resident(nc, wpool, w, f"{pfx}w", split0=True)
        for s in range(KT):
            if s + 1 < KT:
                pend[s + 1] = _load_kblock(nc, xpool, x, s + 1, f"{pfx}a")
            a = pend.pop(s)
            for c in range(NCHUNK):
                cs = slice(c * 512, (c + 1) * 512)
                ps = psum.tile([128, 512], F32)
                for k in range(KT):
                    nc.tensor.matmul(ps[:], a[:, k * 128:(k + 1) * 128],
                                     w_t[k][:, cs], start=(k == 0), stop=(k == KT - 1))
                o16 = spool.tile([128, 512], F16, tag="o16")
                nc.vector.tensor_add(o16[:], ps[:], bb[:, cs])
                nc.sync.dma_start(out_f[s * 128:(s + 1) * 128, cs], o16[:])


def _proj_cols(nc, tc, psum, x, w, bias_part, out_f):
    """kT-style projection: out[e, s] = sum_d W^T[d, e] * X^T[d, s] + bias[e].
    Stationary = weight k-blocks, moving = resident activations."""
    with (
        tc.tile_pool(name="pc_x", bufs=1) as xpool,
        tc.tile_pool(name="pc_w", bufs=3) as wpool,
        tc.tile_pool(name="pc_s", bufs=4) as spool,
        tc.tile_pool(name="pc_b", bufs=1) as bpool,
    ):
        pend = {0: _load_kblock(nc, wpool, w, 0, "kg")}
        bp = bpool.tile([128, KT], F32, tag="biasp")
        nc.sync.dma_start(bp[:], bias_part[:])
        x_t = _load_resident(nc, xpool, x, "kx")
        for e in range(KT):
            if e + 1 < KT:
                pend[e + 1] = _load_kblock(nc, wpool, w, e + 1, "kg")
            g = pend.pop(e)
            for c in range(NCHUNK):
                cs = slice(c * 512, (c + 1) * 512)
                ps = psum.tile([128, 512], F32)
                for k in range(KT):
                    nc.tensor.matmul(ps[:], g[:, k * 128:(k + 1) * 128],
                                     x_t[k][:, cs], start=(k == 0), stop=(k == KT - 1))
                o16 = spool.tile([128, 512], F16, tag="o16")
                nc.vector.tensor_scalar_add(o16[:], ps[:], bp[:, e:e + 1])
                nc.sync.dma_start(out_f[e * 128:(e + 1) * 128, cs], o16[:])


LAG = 6                   # row-tiles attn@v trails behind scores
GRP = 4                   # row-tiles per transpose group


def _attention(nc, tc, psum, q_f, k_f, v_f, at_f, out):
    """Interleaved scores+softmax+attn@v, one 128-row attn block at a time.

    scores[i, j] = sum_m q[m, i]*kT[m, j]; row softmax -> a16 -> at_f DRAM.
    Every GRP row-tiles, 16 [GRP*128, 128] DRAM->SBUF transposes (scalar
    HWDGE) build attn^T group tiles (j on partitions). attn@v for row-tile
    i runs LAG row-tiles behind: ov[i, n] = sum_j attn[i, j] * v[j, n],
    with resident v row-blocks as the moving operand. out is written in
    [i, n] orientation; the host transposes.
    """
    with (
        tc.tile_pool(name="sc_k", bufs=1) as kpool,
        tc.tile_pool(name="av_v", bufs=1) as vpool,
        tc.tile_pool(name="sc_a", bufs=2) as apool,
        tc.tile_pool(name="sc_q", bufs=3) as qpool,
        tc.tile_pool(name="sc_c", bufs=2) as cpool,
        tc.tile_pool(name="sc_s", bufs=2) as spool,
        tc.tile_pool(name="sc_t", bufs=4) as tpool,
        tc.tile_pool(name="sc_r", bufs=8) as rpool,
        tc.tile_pool(name="av_s", bufs=4) as opool,
    ):
        k_t = _load_resident(nc, kpool, k_f, "sk")
        v_r = _load_resident(nc, vpool, v_f, "vr")
        attg_of = {}
        rcp_of = {}

        def emit_av(i):
            attg = attg_of[i // GRP]
            ts = slice((i % GRP) * 128, (i % GRP) * 128 + 128)
            rcp = rcp_of.pop(i)
            for c in range(NCHUNK):
                cs = slice(c * 512, (c + 1) * 512)
                ps = psum.tile([128, 512], F32)
                for j in range(KT):
                    nc.tensor.matmul(ps[:], attg[j][:, ts],
                                     v_r[j][:, cs], start=(j == 0), stop=(j == KT - 1))
                # attn rows are stored unnormalized; fold the softmax
                # 1/rowsum into the output copy (per-partition scalar)
                o16 = opool.tile([128, 512], F16, tag="o16")
                nc.vector.tensor_scalar_mul(o16[:], ps[:], rcp[:])
                nc.sync.dma_start(out[i * 128:(i + 1) * 128, cs], o16[:])

        qpend = {0: _load_kblock(nc, qpool, q_f, 0, "sq"),
                 1: _load_kblock(nc, qpool, q_f, 1, "sq")}
        for i in range(KT):
            if i + 2 < KT:
                qpend[i + 2] = _load_kblock(nc, qpool, q_f, i + 2, "sq")
            qh = qpend.pop(i)
            m4 = tpool.tile([128, NCHUNK], F32, tag="m4")
            sc32 = cpool.tile([128, N], F32, tag="sc32")
            for c in range(NCHUNK):
                cs = slice(c * 512, (c + 1) * 512)
                ps = psum.tile([128, 512], F32)
                for k in range(KT):
                    nc.tensor.matmul(ps[:], qh[:, k * 128:(k + 1) * 128],
                                     k_t[k][:, cs], start=(k == 0), stop=(k == KT - 1))
                nc.vector.reduce_max(m4[:, c:c + 1], ps[:], axis=AX)
                # drain the PSUM bank immediately so scores never starve
                # on banks even when the scalar engine runs behind
                nc.vector.tensor_copy(sc32[:, cs], ps[:])
            mx = tpool.tile([128, 1], F32, tag="mx")
            nc.vector.reduce_max(mx[:], m4[:], axis=AX)
            negm = tpool.tile([128, 1], F32, tag="negm")
            nc.scalar.mul(negm[:], mx[:], -1.0)
            a16 = spool.tile([128, N], F16, tag="a16")
            sume = tpool.tile([128, NCHUNK], F32, tag="sume")
            for c in range(NCHUNK):
                cs = slice(c * 512, (c + 1) * 512)
                nc.scalar.activation(a16[:, cs], sc32[:, cs], EXP,
                                     bias=negm[:], scale=1.0,
                                     accum_out=sume[:, c:c + 1])
            tot = tpool.tile([128, 1], F32, tag="tot")
            nc.vector.reduce_sum(tot[:], sume[:], axis=AX)
            rcp = rpool.tile([128, 1], F32, tag="rcp")
            nc.vector.reciprocal(rcp[:], tot[:])
            rcp_of[i] = rcp
            nc.sync.dma_start(at_f[i * 128:(i + 1) * 128, :], a16[:])
            if i % GRP == GRP - 1:
                g = i // GRP
                attg = [apool.tile([128, GRP * 128], F16, tag=f"ag{j}",
                                   name=f"ag{j}") for j in range(KT)]
                for j in range(KT):
                    nc.scalar.dma_start_transpose(
                        attg[j][:],
                        at_f[g * GRP * 128:(g + 1) * GRP * 128,
                             j * 128:(j + 1) * 128])
                attg_of[g] = attg
            if i >= LAG:
                emit_av(i - LAG)
        for i in range(KT - LAG, KT):
            emit_av(i)


def prepare_in_maps(query, key_, value, Wq, bq, Wk, bk, Wv, bv):
    query = np.asarray(query, dtype=np.float32)
    key_ = np.asarray(key_, dtype=np.float32)
    value = np.asarray(value, dtype=np.float32)
    Wq = np.asarray(Wq, dtype=np.float32)
    Wk = np.asarray(Wk, dtype=np.float32)
    Wv = np.asarray(Wv, dtype=np.float32)
    bq = np.asarray(bq, dtype=np.float32)
    bk = np.asarray(bk, dtype=np.float32)
    bv = np.asarray(bv, dtype=np.float32)

    wqt = np.ascontiguousarray(Wq.T).astype(np.float16)
    wkt = np.ascontiguousarray(Wk.T).astype(np.float16)
    wvt = np.ascontiguousarray(Wv.T).astype(np.float16)
    bqb = np.broadcast_to(bq, (128, N)).copy()
    bvb = np.broadcast_to(bv, (128, N)).copy()
    bkp = np.ascontiguousarray(bk.reshape(KT, 128).T)

    in_maps = []
    for b in range(B):
        in_maps.append({
            "xq": np.ascontiguousarray(query[b].T).astype(np.float16),
            "xk": np.ascontiguousarray(key_[b].T).astype(np.float16),
            "xv": np.ascontiguousarray(value[b].T).astype(np.float16),
            "wq": wqt, "wk": wkt, "wv": wvt,
            "bqb": bqb, "bkp": bkp, "bvb": bvb,
        })
    return in_maps


def get_nc():
    if "nc" not in _compiled:
        _compiled["nc"] = _build()
    return _compiled["nc"]


def kernel(query, key_, value, Wq, bq, Wk, bk, Wv, bv):
    in_maps = prepare_in_maps(query, key_, value, Wq, bq, Wk, bk, Wv, bv)
    res = run_bass_kernel_spmd(get_nc(), in_maps, core_ids=list(range(B)))
    # device emits (attn @ v)[i, n]; the module's output is its transpose
    return np.stack([np.asarray(res.results[b]["out"]).T for b in range(B)]).astype(np.float32)


if __name__ == "__main__":
    rng = np.random.default_rng(0)
    inputs = {
        "query": rng.standard_normal((B, S, D), dtype=np.float32),
        "key_": rng.standard_normal((B, S, D), dtype=np.float32),
        "value": rng.standard_normal((B, S, D), dtype=np.float32),
        "Wq": (rng.standard_normal((D, D), dtype=np.float32) / np.sqrt(D)),
        "bq": rng.standard_normal(D).astype(np.float32) * 0.01,
        "Wk": (rng.standard_normal((D, D), dtype=np.float32) / np.sqrt(D)),
        "bk": rng.standard_normal(D).astype(np.float32) * 0.01,
        "Wv": (rng.standard_normal((D, D), dtype=np.float32) / np.sqrt(D)),
        "bv": rng.standard_normal(D).astype(np.float32) * 0.01,
    }
    out = kernel(**inputs)
    print("out", out.shape, out.dtype)


# revision 22
# speedup vs baseline: 1.0820x; 1.0056x over previous
"""Trainium2 Bass kernel for nn_Model_39676907883957 (dense_transformer).

Math (per batch element b, with S = D = N = 2048):
    q = Xq @ Wq^T + bq            # [S, D]
    kT = Wk @ Xk^T + bk[:, None]  # [D, S]  (k projected directly in transposed layout)
    v = Xv @ Wv^T + bv            # [S, D]
    scores[i, j] = sum_m q[m, i] * kT[m, j]          # q^T @ k^T
    attn = softmax_rows(scores)
    out[n, i] = sum_j v[j, n] * attn[i, j]           # == (attn @ v)^T

Sharding: data-parallel over batch, B=8 -> one batch element per NeuronCore.

Numerics: all matmuls on the PE in fp16 single-pass with fp32 PSUM
accumulation (measured rel err ~3.7e-3 vs the fp64 reference, against a
2e-2 gate). 5 x 2048^3 MACs per core.
"""

import numpy as np

import concourse.bass as bass
import concourse.bacc as bacc
import concourse.tile as tile
import concourse.mybir as mybir
from concourse.bass_utils import run_bass_kernel_spmd

B, S, D = 8, 2048, 2048
N = 2048                 # S == D
KT = N // 128            # 16 contraction tiles
NCHUNK = N // 512        # 4 free-dim chunks of 512
F16 = mybir.dt.float16
F32 = mybir.dt.float32
AX = mybir.AxisListType.X
EXP = mybir.ActivationFunctionType.Exp

_compiled = {}


def _build():
    nc = bacc.Bacc("TRN2", target_bir_lowering=False, debug=False)

    # ExternalInputs (per core). x* are host-transposed activations [d, s] fp16.
    xq = nc.dram_tensor("xq", [N, N], F16, kind="ExternalInput").ap()
    xk = nc.dram_tensor("xk", [N, N], F16, kind="ExternalInput").ap()
    xv = nc.dram_tensor("xv", [N, N], F16, kind="ExternalInput").ap()
    # host-transposed weights [d, e] fp16
    wq = nc.dram_tensor("wq", [N, N], F16, kind="ExternalInput").ap()
    wk = nc.dram_tensor("wk", [N, N], F16, kind="ExternalInput").ap()
    wv = nc.dram_tensor("wv", [N, N], F16, kind="ExternalInput").ap()
    # biases: bqb/bvb broadcast across partitions [128, N]; bkp partition-major [128, 16]
    bqb = nc.dram_tensor("bqb", [128, N], F32, kind="ExternalInput").ap()
    bkp = nc.dram_tensor("bkp", [128, KT], F32, kind="ExternalInput").ap()
    bvb = nc.dram_tensor("bvb", [128, N], F32, kind="ExternalInput").ap()

    out = nc.dram_tensor("out", [N, N], F16, kind="ExternalOutput").ap()

    with tile.TileContext(nc, pool_alloc_mode="queue") as tc:
        with tc.tile_pool(name="dram", bufs=1, space="DRAM") as dram:
            q_f = dram.tile([N, N], F16, tag="q_f")
            k_f = dram.tile([N, N], F16, tag="k_f")
            v_f = dram.tile([N, N], F16, tag="v_f")
            at_f = dram.tile([N, N], F16, tag="at_f")

            with tc.tile_pool(name="psum", bufs=8, space="PSUM") as psum:
                _proj_rows(nc, tc, psum, xq, wq, bqb, q_f, "q", cold=True)
                _proj_cols(nc, tc, psum, xk, wk, bkp, k_f)
                _proj_rows(nc, tc, psum, xv, wv, bvb, v_f, "v")
                _attention(nc, tc, psum, q_f, k_f, v_f, at_f, out)

    nc.compile()
    return nc


def _load_kblock(nc, pool, dram_ap, col_blk, tag, dt=F16):
    """Load DRAM[:, col_blk*128 : +128] ([N, 128]) into one [128, N] SBUF tile
    whose slice [:, k*128:(k+1)*128] is contraction-tile k (partition = row%128)."""
    t = pool.tile([128, N], dt, tag=tag)
    src = dram_ap[:, col_blk * 128:(col_blk + 1) * 128].rearrange(
        "(t p) s -> p t s", p=128
    )
    dst = t[:].rearrange("p (t s) -> p t s", t=KT)
    nc.sync.dma_start(dst, src)
    return t


def _load_resident(nc, pool, src, tag, split0=False):
    """Load an [N, N] fp16 DRAM tensor as KT resident [128, N] row-block tiles,
    chunk-0 columns first, on the SWDGE path (separate from HWDGE streaming).
    split0 puts half of chunk 0 on the sync path to halve cold-start latency."""
    ts = [pool.tile([128, N], F16, tag=f"{tag}{k}", name=f"{tag}{k}") for k in range(KT)]
    for c in range(NCHUNK):
        cs = slice(c * 512, (c + 1) * 512)
        for k in range(KT):
            eng = nc.sync if (split0 and c == 0 and k % 2 == 1) else nc.gpsimd
            eng.dma_start(ts[k][:, cs], src[k * 128:(k + 1) * 128, cs])
    return ts


def _proj_rows(nc, tc, psum, x, w, bias_bcast, out_f, pfx, cold=False):
    """Row-major projection: out[s, e] = sum_d X^T[d, s] * W^T[d, e] + bias[e].
    Stationary = activation k-blocks, moving = resident weights.
    cold: run the first two row-tiles chunk-major so the per-chunk demand
    on the streaming weight loads is halved while HBM ramps up."""
    with (
        tc.tile_pool(name=f"p{pfx}_w", bufs=1) as wpool,
        tc.tile_pool(name=f"p{pfx}_x", bufs=4) as xpool,
        tc.tile_pool(name=f"p{pfx}_s", bufs=4) as spool,
        tc.tile_pool(name=f"p{pfx}_b", bufs=1) as bpool,
    ):
        pend = {0: _load_kblock(nc, xpool, x, 0, f"{pfx}a")}
        if cold:
            pend[1] = _load_kblock(nc, xpool, x, 1, f"{pfx}a")
        bb = bpool.tile([128, N], F32, tag="bias")
        nc.sync.dma_start(bb[:], bias_bcast[:])
        w_t = _load_resident(nc, wpool, w, f"{pfx}w", split0=True)

        def row_chunk(a, s, c):
            cs = slice(c * 512, (c + 1) * 512)
            ps = psum.tile([128, 512], F32)
            for k in range(KT):
                nc.tensor.matmul(ps[:], a[:, k * 128:(k + 1) * 128],
                                 w_t[k][:, cs], start=(k == 0), stop=(k == KT - 1))
            o16 = spool.tile([128, 512], F16, tag="o16")
            nc.vector.tensor_add(o16[:], ps[:], bb[:, cs])
            nc.sync.dma_start(out_f[s * 128:(s + 1) * 128, cs], o16[:])

        start_s = 0
        if cold:
            a0, a1 = pend.pop(0), pend.pop(1)
            pend[2] = _load_kblock(nc, xpool, x, 2, f"{pfx}a")
            for c in range(NCHUNK):
                row_chunk(a0, 0, c)
                row_chunk(a1, 1, c)
            start_s = 2
        for s in range(start_s, KT):
            if s + 1 < KT:
                pend[s + 1] = _load_kblock(nc, xpool, x, s + 1, f"{pfx}a")
            a = pend.pop(s)
            for c in range(NCHUNK):
                row_chunk(a, s, c)


def _proj_cols(nc, tc, psum, x, w, bias_part, out_f):
    """kT-style projection: out[e, s] = sum_d W^T[d, e] * X^T[d, s] + bias[e].
    Stationary = weight k-blocks, moving = resident activations."""
    with (
        tc.tile_pool(name="pc_x", bufs=1) as xpool,
        tc.tile_pool(name="pc_w", bufs=3) as wpool,
        tc.tile_pool(name="pc_s", bufs=4) as spool,
        tc.tile_pool(name="pc_b", bufs=1) as bpool,
    ):
        pend = {0: _load_kblock(nc, wpool, w, 0, "kg")}
        bp = bpool.tile([128, KT], F32, tag="biasp")
        nc.sync.dma_start(bp[:], bias_part[:])
        x_t = _load_resident(nc, xpool, x, "kx")
        for e in range(KT):
            if e + 1 < KT:
                pend[e + 1] = _load_kblock(nc, wpool, w, e + 1, "kg")
            g = pend.pop(e)
            for c in range(NCHUNK):
                cs = slice(c * 512, (c + 1) * 512)
                ps = psum.tile([128, 512], F32)
                for k in range(KT):
                    nc.tensor.matmul(ps[:], g[:, k * 128:(k + 1) * 128],
                                     x_t[k][:, cs], start=(k == 0), stop=(k == KT - 1))
                o16 = spool.tile([128, 512], F16, tag="o16")
                nc.vector.tensor_scalar_add(o16[:], ps[:], bp[:, e:e + 1])
                nc.sync.dma_start(out_f[e * 128:(e + 1) * 128, cs], o16[:])


LAG = 6                   # row-tiles attn@v trails behind scores
GRP = 4                   # row-tiles per transpose group


def _attention(nc, tc, psum, q_f, k_f, v_f, at_f, out):
    """Interleaved scores+softmax+attn@v, one 128-row attn block at a time.

    scores[i, j] = sum_m q[m, i]*kT[m, j]; row softmax -> a16 -> at_f DRAM.
    Every GRP row-tiles, 16 [GRP*128, 128] DRAM->SBUF transposes (scalar
    HWDGE) build attn^T group tiles (j on partitions). attn@v for row-tile
    i runs LAG row-tiles behind: ov[i, n] = sum_j attn[i, j] * v[j, n],
    with resident v row-blocks as the moving operand. out is written in
    [i, n] orientation; the host transposes.
    """
    with (
        tc.tile_pool(name="sc_k", bufs=1) as kpool,
        tc.tile_pool(name="av_v", bufs=1) as vpool,
        tc.tile_pool(name="sc_a", bufs=2) as apool,
        tc.tile_pool(name="sc_q", bufs=3) as qpool,
        tc.tile_pool(name="sc_c", bufs=2) as cpool,
        tc.tile_pool(name="sc_s", bufs=2) as spool,
        tc.tile_pool(name="sc_t", bufs=4) as tpool,
        tc.tile_pool(name="sc_r", bufs=8) as rpool,
        tc.tile_pool(name="av_s", bufs=4) as opool,
    ):
        k_t = _load_resident(nc, kpool, k_f, "sk")
        v_r = _load_resident(nc, vpool, v_f, "vr")
        attg_of = {}
        rcp_of = {}

        def emit_av(i):
            attg = attg_of[i // GRP]
            ts = slice((i % GRP) * 128, (i % GRP) * 128 + 128)
            rcp = rcp_of.pop(i)
            for c in range(NCHUNK):
                cs = slice(c * 512, (c + 1) * 512)
                ps = psum.tile([128, 512], F32)
                for j in range(KT):
                    nc.tensor.matmul(ps[:], attg[j][:, ts],
                                     v_r[j][:, cs], start=(j == 0), stop=(j == KT - 1))
                # attn rows are stored unnormalized; fold the softmax
                # 1/rowsum into the output copy (per-partition scalar)
                o16 = opool.tile([128, 512], F16, tag="o16")
                nc.vector.tensor_scalar_mul(o16[:], ps[:], rcp[:])
                nc.sync.dma_start(out[i * 128:(i + 1) * 128, cs], o16[:])

        qpend = {0: _load_kblock(nc, qpool, q_f, 0, "sq"),
                 1: _load_kblock(nc, qpool, q_f, 1, "sq")}
        for i in range(KT):
            if i + 2 < KT:
                qpend[i + 2] = _load_kblock(nc, qpool, q_f, i + 2, "sq")
            qh = qpend.pop(i)
            m4 = tpool.tile([128, NCHUNK], F32, tag="m4")
            sc32 = cpool.tile([128, N], F32, tag="sc32")
            for c in range(NCHUNK):
                cs = slice(c * 512, (c + 1) * 512)
                ps = psum.tile([128, 512], F32)
                for k in range(KT):
                    nc.tensor.matmul(ps[:], qh[:, k * 128:(k + 1) * 128],
                                     k_t[k][:, cs], start=(k == 0), stop=(k == KT - 1))
                nc.vector.reduce_max(m4[:, c:c + 1], ps[:], axis=AX)
                # drain the PSUM bank immediately so scores never starve
                # on banks even when the scalar engine runs behind
                nc.vector.tensor_copy(sc32[:, cs], ps[:])
            mx = tpool.tile([128, 1], F32, tag="mx")
            nc.vector.reduce_max(mx[:], m4[:], axis=AX)
            negm = tpool.tile([128, 1], F32, tag="negm")
            nc.scalar.mul(negm[:], mx[:], -1.0)
            a16 = spool.tile([128, N], F16, tag="a16")
            sume = tpool.tile([128, NCHUNK], F32, tag="sume")
            for c in range(NCHUNK):
                cs = slice(c * 512, (c + 1) * 512)
                nc.scalar.activation(a16[:, cs], sc32[:, cs], EXP,
                                     bias=negm[:], scale=1.0,
                                     accum_out=sume[:, c:c + 1])
            tot = tpool.tile([128, 1], F32, tag="tot")
            nc.vector.reduce_sum(tot[:], sume[:], axis=AX)
            rcp = rpool.tile([128, 1], F32, tag="rcp")
            nc.vector.reciprocal(rcp[:], tot[:])
            rcp_of[i] = rcp
            nc.sync.dma_start(at_f[i * 128:(i + 1) * 128, :], a16[:])
            if i % GRP == GRP - 1:
                g = i // GRP
                attg = [apool.tile([128, GRP * 128], F16, tag=f"ag{j}",
                                   name=f"ag{j}") for j in range(KT)]
                for j in range(KT):
                    nc.scalar.dma_start_transpose(
                        attg[j][:],
                        at_f[g * GRP * 128:(g + 1) * GRP * 128,
                             j * 128:(j + 1) * 128])
                attg_of[g] = attg
            if i >= LAG:
                emit_av(i - LAG)
        for i in range(KT - LAG, KT):
            emit_av(i)


def prepare_in_maps(query, key_, value, Wq, bq, Wk, bk, Wv, bv):
    query = np.asarray(query, dtype=np.float32)
    key_ = np.asarray(key_, dtype=np.float32)
    value = np.asarray(value, dtype=np.float32)
    Wq = np.asarray(Wq, dtype=np.float32)
    Wk = np.asarray(Wk, dtype=np.float32)
    Wv = np.asarray(Wv, dtype=np.float32)
    bq = np.asarray(bq, dtype=np.float32)
    bk = np.asarray(bk, dtype=np.float32)
    bv = np.asarray(bv, dtype=np.float32)

    wqt = np.ascontiguousarray(Wq.T).astype(np.float16)
    wkt = np.ascontiguousarray(Wk.T).astype(np.float16)
    wvt = np.ascontiguousarray(Wv.T).astype(np.float16)
    bqb = np.broadcast_to(bq, (128, N)).copy()
    bvb = np.broadcast_to(bv, (128, N)).copy()
    bkp = np.ascontiguousarray(bk.reshape(KT, 128).T)

    in_maps = []
    for b in range(B):
        in_maps.append({
            "xq": np.ascontiguousarray(query[b].T).astype(np.float16),
            "xk": np.ascontiguousarray(key_[b].T).astype(np.float16),
            "xv": np.ascontiguousarray(value[b].T).astype(np.float16),
            "wq": wqt, "wk": wkt, "wv": wvt,
            "bqb": bqb, "bkp": bkp, "bvb": bvb,
        })
    return in_maps


def get_nc():
    if "nc" not in _compiled:
        _compiled["nc"] = _build()
    return _compiled["nc"]


def kernel(query, key_, value, Wq, bq, Wk, bk, Wv, bv):
    in_maps = prepare_in_maps(query, key_, value, Wq, bq, Wk, bk, Wv, bv)
    res = run_bass_kernel_spmd(get_nc(), in_maps, core_ids=list(range(B)))
    # device emits (attn @ v)[i, n]; the module's output is its transpose
    return np.stack([np.asarray(res.results[b]["out"]).T for b in range(B)]).astype(np.float32)


if __name__ == "__main__":
    rng = np.random.default_rng(0)
    inputs = {
        "query": rng.standard_normal((B, S, D), dtype=np.float32),
        "key_": rng.standard_normal((B, S, D), dtype=np.float32),
        "value": rng.standard_normal((B, S, D), dtype=np.float32),
        "Wq": (rng.standard_normal((D, D), dtype=np.float32) / np.sqrt(D)),
        "bq": rng.standard_normal(D).astype(np.float32) * 0.01,
        "Wk": (rng.standard_normal((D, D), dtype=np.float32) / np.sqrt(D)),
        "bk": rng.standard_normal(D).astype(np.float32) * 0.01,
        "Wv": (rng.standard_normal((D, D), dtype=np.float32) / np.sqrt(D)),
        "bv": rng.standard_normal(D).astype(np.float32) * 0.01,
    }
    out = kernel(**inputs)
    print("out", out.shape, out.dtype)


# revision 27
# speedup vs baseline: 1.0867x; 1.0043x over previous
"""Trainium2 Bass kernel for nn_Model_39676907883957 (dense_transformer).

Math (per batch element b, with S = D = N = 2048):
    q = Xq @ Wq^T + bq            # [S, D]
    kT = Wk @ Xk^T + bk[:, None]  # [D, S]  (k projected directly in transposed layout)
    v = Xv @ Wv^T + bv            # [S, D]
    scores[i, j] = sum_m q[m, i] * kT[m, j]          # q^T @ k^T
    attn = softmax_rows(scores)
    out[n, i] = sum_j v[j, n] * attn[i, j]           # == (attn @ v)^T

Sharding: data-parallel over batch, B=8 -> one batch element per NeuronCore.

Numerics: all matmuls on the PE in fp16 single-pass with fp32 PSUM
accumulation (measured rel err ~3.7e-3 vs the fp64 reference, against a
2e-2 gate). 5 x 2048^3 MACs per core.
"""

import numpy as np

import concourse.bass as bass
import concourse.bacc as bacc
import concourse.tile as tile
import concourse.mybir as mybir
from concourse.bass_utils import run_bass_kernel_spmd

B, S, D = 8, 2048, 2048
N = 2048                 # S == D
KT = N // 128            # 16 contraction tiles
NCHUNK = N // 512        # 4 free-dim chunks of 512
F16 = mybir.dt.float16
F32 = mybir.dt.float32
AX = mybir.AxisListType.X
EXP = mybir.ActivationFunctionType.Exp

_compiled = {}


def _build():
    nc = bacc.Bacc("TRN2", target_bir_lowering=False, debug=False)

    # ExternalInputs (per core). x* are host-transposed activations [d, s] fp16.
    xq = nc.dram_tensor("xq", [N, N], F16, kind="ExternalInput").ap()
    xk = nc.dram_tensor("xk", [N, N], F16, kind="ExternalInput").ap()
    xv = nc.dram_tensor("xv", [N, N], F16, kind="ExternalInput").ap()
    # host-transposed weights [d, e] fp16
    wq = nc.dram_tensor("wq", [N, N], F16, kind="ExternalInput").ap()
    wk = nc.dram_tensor("wk", [N, N], F16, kind="ExternalInput").ap()
    wv = nc.dram_tensor("wv", [N, N], F16, kind="ExternalInput").ap()
    # biases: bqb/bvb broadcast across partitions [128, N]; bkp partition-major [128, 16]
    bqb = nc.dram_tensor("bqb", [128, N], F32, kind="ExternalInput").ap()
    bkp = nc.dram_tensor("bkp", [128, KT], F32, kind="ExternalInput").ap()
    bvb = nc.dram_tensor("bvb", [128, N], F32, kind="ExternalInput").ap()

    out = nc.dram_tensor("out", [N, N], F16, kind="ExternalOutput").ap()

    with tile.TileContext(nc, pool_alloc_mode="queue") as tc:
        with tc.tile_pool(name="dram", bufs=1, space="DRAM") as dram:
            q_f = dram.tile([N, N], F16, tag="q_f")
            k_f = dram.tile([N, N], F16, tag="k_f")
            v_f = dram.tile([N, N], F16, tag="v_f")
            at_f = dram.tile([N, N], F16, tag="at_f")

            with tc.tile_pool(name="psum", bufs=8, space="PSUM") as psum:
                _proj_rows(nc, tc, psum, xq, wq, bqb, q_f, "q", cold=True)
                _proj_cols(nc, tc, psum, xk, wk, bkp, k_f)
                _proj_rows(nc, tc, psum, xv, wv, bvb, v_f, "v")
                _attention(nc, tc, psum, q_f, k_f, v_f, at_f, out)

    nc.compile()
    return nc


def _load_kblock(nc, pool, dram_ap, col_blk, tag, dt=F16, parts=1):
    """Load DRAM[:, col_blk*128 : +128] ([N, 128]) into one [128, N] SBUF tile
    whose slice [:, k*128:(k+1)*128] is contraction-tile k (partition = row%128).
    parts>1 splits the transfer across that many DMA queues (cold start)."""
    t = pool.tile([128, N], dt, tag=tag)
    seg = N // parts
    for h in range(parts):
        src = dram_ap[h * seg:(h + 1) * seg,
                      col_blk * 128:(col_blk + 1) * 128].rearrange(
            "(t p) s -> p t s", p=128
        )
        dst = t[:, h * seg:(h + 1) * seg].rearrange(
            "p (t s) -> p t s", t=KT // parts)
        nc.sync.dma_start(dst, src)
    return t


def _load_resident(nc, pool, src, tag, split0=False):
    """Load an [N, N] fp16 DRAM tensor as KT resident [128, N] row-block tiles,
    chunk-0 columns first, on the SWDGE path (separate from HWDGE streaming).
    split0 puts half of chunk 0 on the sync path to halve cold-start latency."""
    ts = [pool.tile([128, N], F16, tag=f"{tag}{k}", name=f"{tag}{k}") for k in range(KT)]
    for c in range(NCHUNK):
        cs = slice(c * 512, (c + 1) * 512)
        for k in range(KT):
            eng = nc.sync if (split0 and c == 0 and k % 2 == 1) else nc.gpsimd
            eng.dma_start(ts[k][:, cs], src[k * 128:(k + 1) * 128, cs])
    return ts


def _proj_rows(nc, tc, psum, x, w, bias_bcast, out_f, pfx, cold=False):
    """Row-major projection: out[s, e] = sum_d X^T[d, s] * W^T[d, e] + bias[e].
    Stationary = activation k-blocks, moving = resident weights.
    cold: run the first two row-tiles chunk-major so the per-chunk demand
    on the streaming weight loads is halved while HBM ramps up."""
    with (
        tc.tile_pool(name=f"p{pfx}_w", bufs=1) as wpool,
        tc.tile_pool(name=f"p{pfx}_x", bufs=6) as xpool,
        tc.tile_pool(name=f"p{pfx}_s", bufs=4) as spool,
        tc.tile_pool(name=f"p{pfx}_b", bufs=1) as bpool,
    ):
        if cold:
            pend = {s: _load_kblock(nc, xpool, x, s, f"{pfx}a", parts=p)
                    for s, p in enumerate((4, 2, 2, 1))}
            w_t = _load_resident(nc, wpool, w, f"{pfx}w", split0=True)
            bb = bpool.tile([128, N], F32, tag="bias")
            nc.sync.dma_start(bb[:], bias_bcast[:])
        else:
            pend = {0: _load_kblock(nc, xpool, x, 0, f"{pfx}a")}
            bb = bpool.tile([128, N], F32, tag="bias")
            nc.sync.dma_start(bb[:], bias_bcast[:])
            w_t = _load_resident(nc, wpool, w, f"{pfx}w", split0=True)

        def row_chunk(a, s, c):
            cs = slice(c * 512, (c + 1) * 512)
            ps = psum.tile([128, 512], F32)
            for k in range(KT):
                nc.tensor.matmul(ps[:], a[:, k * 128:(k + 1) * 128],
                                 w_t[k][:, cs], start=(k == 0), stop=(k == KT - 1))
            o16 = spool.tile([128, 512], F16, tag="o16")
            nc.vector.tensor_add(o16[:], ps[:], bb[:, cs])
            nc.sync.dma_start(out_f[s * 128:(s + 1) * 128, cs], o16[:])

        start_s = 0
        if cold:
            colds = [pend.pop(s) for s in range(4)]
            pend[4] = _load_kblock(nc, xpool, x, 4, f"{pfx}a")
            for c in range(NCHUNK):
                for s, a in enumerate(colds):
                    row_chunk(a, s, c)
            start_s = 4
        for s in range(start_s, KT):
            if s + 1 < KT:
                pend[s + 1] = _load_kblock(nc, xpool, x, s + 1, f"{pfx}a")
            a = pend.pop(s)
            for c in range(NCHUNK):
                row_chunk(a, s, c)


def _proj_cols(nc, tc, psum, x, w, bias_part, out_f):
    """kT-style projection: out[e, s] = sum_d W^T[d, e] * X^T[d, s] + bias[e].
    Stationary = weight k-blocks, moving = resident activations."""
    with (
        tc.tile_pool(name="pc_x", bufs=1) as xpool,
        tc.tile_pool(name="pc_w", bufs=3) as wpool,
        tc.tile_pool(name="pc_s", bufs=4) as spool,
        tc.tile_pool(name="pc_b", bufs=1) as bpool,
    ):
        pend = {0: _load_kblock(nc, wpool, w, 0, "kg")}
        bp = bpool.tile([128, KT], F32, tag="biasp")
        nc.sync.dma_start(bp[:], bias_part[:])
        x_t = _load_resident(nc, xpool, x, "kx")
        for e in range(KT):
            if e + 1 < KT:
                pend[e + 1] = _load_kblock(nc, wpool, w, e + 1, "kg")
            g = pend.pop(e)
            for c in range(NCHUNK):
                cs = slice(c * 512, (c + 1) * 512)
                ps = psum.tile([128, 512], F32)
                for k in range(KT):
                    nc.tensor.matmul(ps[:], g[:, k * 128:(k + 1) * 128],
                                     x_t[k][:, cs], start=(k == 0), stop=(k == KT - 1))
                o16 = spool.tile([128, 512], F16, tag="o16")
                nc.vector.tensor_scalar_add(o16[:], ps[:], bp[:, e:e + 1])
                nc.sync.dma_start(out_f[e * 128:(e + 1) * 128, cs], o16[:])


LAG = 6                   # row-tiles attn@v trails behind scores
GRP = 4                   # row-tiles per transpose group


def _attention(nc, tc, psum, q_f, k_f, v_f, at_f, out):
    """Interleaved scores+softmax+attn@v, one 128-row attn block at a time.

    scores[i, j] = sum_m q[m, i]*kT[m, j]; row softmax -> a16 -> at_f DRAM.
    Every GRP row-tiles, 16 [GRP*128, 128] DRAM->SBUF transposes (scalar
    HWDGE) build attn^T group tiles (j on partitions). attn@v for row-tile
    i runs LAG row-tiles behind: ov[i, n] = sum_j attn[i, j] * v[j, n],
    with resident v row-blocks as the moving operand. out is written in
    [i, n] orientation; the host transposes.
    """
    with (
        tc.tile_pool(name="sc_k", bufs=1) as kpool,
        tc.tile_pool(name="av_v", bufs=1) as vpool,
        tc.tile_pool(name="sc_a", bufs=2) as apool,
        tc.tile_pool(name="sc_q", bufs=3) as qpool,
        tc.tile_pool(name="sc_c", bufs=2) as cpool,
        tc.tile_pool(name="sc_s", bufs=2) as spool,
        tc.tile_pool(name="sc_t", bufs=4) as tpool,
        tc.tile_pool(name="sc_r", bufs=8) as rpool,
        tc.tile_pool(name="av_s", bufs=4) as opool,
    ):
        k_t = _load_resident(nc, kpool, k_f, "sk")
        v_r = _load_resident(nc, vpool, v_f, "vr")
        attg_of = {}
        rcp_of = {}

        def emit_av(i):
            attg = attg_of[i // GRP]
            ts = slice((i % GRP) * 128, (i % GRP) * 128 + 128)
            rcp = rcp_of.pop(i)
            for c in range(NCHUNK):
                cs = slice(c * 512, (c + 1) * 512)
                ps = psum.tile([128, 512], F32)
                for j in range(KT):
                    nc.tensor.matmul(ps[:], attg[j][:, ts],
                                     v_r[j][:, cs], start=(j == 0), stop=(j == KT - 1))
                # attn rows are stored unnormalized; fold the softmax
                # 1/rowsum into the output copy (per-partition scalar)
                o16 = opool.tile([128, 512], F16, tag="o16")
                nc.vector.tensor_scalar_mul(o16[:], ps[:], rcp[:])
                nc.sync.dma_start(out[i * 128:(i + 1) * 128, cs], o16[:])

        qpend = {0: _load_kblock(nc, qpool, q_f, 0, "sq"),
                 1: _load_kblock(nc, qpool, q_f, 1, "sq")}
        for i in range(KT):
            if i + 2 < KT:
                qpend[i + 2] = _load_kblock(nc, qpool, q_f, i + 2, "sq")
            qh = qpend.pop(i)
            m4 = tpool.tile([128, NCHUNK], F32, tag="m4")
            sc32 = cpool.tile([128, N], F32, tag="sc32")
            for c in range(NCHUNK):
                cs = slice(c * 512, (c + 1) * 512)
                ps = psum.tile([128, 512], F32)
                for k in range(KT):
                    nc.tensor.matmul(ps[:], qh[:, k * 128:(k + 1) * 128],
                                     k_t[k][:, cs], start=(k == 0), stop=(k == KT - 1))
                nc.vector.reduce_max(m4[:, c:c + 1], ps[:], axis=AX)
                # drain the PSUM bank immediately so scores never starve
                # on banks even when the scalar engine runs behind
                nc.vector.tensor_copy(sc32[:, cs], ps[:])
            mx = tpool.tile([128, 1], F32, tag="mx")
            nc.vector.reduce_max(mx[:], m4[:], axis=AX)
            negm = tpool.tile([128, 1], F32, tag="negm")
            nc.scalar.mul(negm[:], mx[:], -1.0)
            a16 = spool.tile([128, N], F16, tag="a16")
            sume = tpool.tile([128, NCHUNK], F32, tag="sume")
            for c in range(NCHUNK):
                cs = slice(c * 512, (c + 1) * 512)
                nc.scalar.activation(a16[:, cs], sc32[:, cs], EXP,
                                     bias=negm[:], scale=1.0,
                                     accum_out=sume[:, c:c + 1])
            tot = tpool.tile([128, 1], F32, tag="tot")
            nc.vector.reduce_sum(tot[:], sume[:], axis=AX)
            rcp = rpool.tile([128, 1], F32, tag="rcp")
            nc.vector.reciprocal(rcp[:], tot[:])
            rcp_of[i] = rcp
            nc.sync.dma_start(at_f[i * 128:(i + 1) * 128, :], a16[:])
            if i % GRP == GRP - 1:
                g = i // GRP
                attg = [apool.tile([128, GRP * 128], F16, tag=f"ag{j}",
                                   name=f"ag{j}") for j in range(KT)]
                for j in range(KT):
                    nc.scalar.dma_start_transpose(
                        attg[j][:],
                        at_f[g * GRP * 128:(g + 1) * GRP * 128,
                             j * 128:(j + 1) * 128])
                attg_of[g] = attg
            if i >= LAG:
                emit_av(i - LAG)
        for i in range(KT - LAG, KT):
            emit_av(i)


def prepare_in_maps(query, key_, value, Wq, bq, Wk, bk, Wv, bv):
    query = np.asarray(query, dtype=np.float32)
    key_ = np.asarray(key_, dtype=np.float32)
    value = np.asarray(value, dtype=np.float32)
    Wq = np.asarray(Wq, dtype=np.float32)
    Wk = np.asarray(Wk, dtype=np.float32)
    Wv = np.asarray(Wv, dtype=np.float32)
    bq = np.asarray(bq, dtype=np.float32)
    bk = np.asarray(bk, dtype=np.float32)
    bv = np.asarray(bv, dtype=np.float32)

    wqt = np.ascontiguousarray(Wq.T).astype(np.float16)
    wkt = np.ascontiguousarray(Wk.T).astype(np.float16)
    wvt = np.ascontiguousarray(Wv.T).astype(np.float16)
    bqb = np.broadcast_to(bq, (128, N)).copy()
    bvb = np.broadcast_to(bv, (128, N)).copy()
    bkp = np.ascontiguousarray(bk.reshape(KT, 128).T)

    in_maps = []
    for b in range(B):
        in_maps.append({
            "xq": np.ascontiguousarray(query[b].T).astype(np.float16),
            "xk": np.ascontiguousarray(key_[b].T).astype(np.float16),
            "xv": np.ascontiguousarray(value[b].T).astype(np.float16),
            "wq": wqt, "wk": wkt, "wv": wvt,
            "bqb": bqb, "bkp": bkp, "bvb": bvb,
        })
    return in_maps


def get_nc():
    if "nc" not in _compiled:
        _compiled["nc"] = _build()
    return _compiled["nc"]


def kernel(query, key_, value, Wq, bq, Wk, bk, Wv, bv):
    in_maps = prepare_in_maps(query, key_, value, Wq, bq, Wk, bk, Wv, bv)
    res = run_bass_kernel_spmd(get_nc(), in_maps, core_ids=list(range(B)))
    # device emits (attn @ v)[i, n]; the module's output is its transpose
    return np.stack([np.asarray(res.results[b]["out"]).T for b in range(B)]).astype(np.float32)


if __name__ == "__main__":
    rng = np.random.default_rng(0)
    inputs = {
        "query": rng.standard_normal((B, S, D), dtype=np.float32),
        "key_": rng.standard_normal((B, S, D), dtype=np.float32),
        "value": rng.standard_normal((B, S, D), dtype=np.float32),
        "Wq": (rng.standard_normal((D, D), dtype=np.float32) / np.sqrt(D)),
        "bq": rng.standard_normal(D).astype(np.float32) * 0.01,
        "Wk": (rng.standard_normal((D, D), dtype=np.float32) / np.sqrt(D)),
        "bk": rng.standard_normal(D).astype(np.float32) * 0.01,
        "Wv": (rng.standard_normal((D, D), dtype=np.float32) / np.sqrt(D)),
        "bv": rng.standard_normal(D).astype(np.float32) * 0.01,
    }
    out = kernel(**inputs)
    print("out", out.shape, out.dtype)


# revision 29
# speedup vs baseline: 1.0939x; 1.0066x over previous
"""Trainium2 Bass kernel for nn_Model_39676907883957 (dense_transformer).

Math (per batch element b, with S = D = N = 2048):
    q = Xq @ Wq^T + bq            # [S, D]
    kT = Wk @ Xk^T + bk[:, None]  # [D, S]  (k projected directly in transposed layout)
    v = Xv @ Wv^T + bv            # [S, D]
    scores[i, j] = sum_m q[m, i] * kT[m, j]          # q^T @ k^T
    attn = softmax_rows(scores)
    out[n, i] = sum_j v[j, n] * attn[i, j]           # == (attn @ v)^T

Sharding: data-parallel over batch, B=8 -> one batch element per NeuronCore.

Numerics: all matmuls on the PE in fp16 single-pass with fp32 PSUM
accumulation (measured rel err ~3.7e-3 vs the fp64 reference, against a
2e-2 gate). 5 x 2048^3 MACs per core.
"""

import numpy as np

import concourse.bass as bass
import concourse.bacc as bacc
import concourse.tile as tile
import concourse.mybir as mybir
from concourse.bass_utils import run_bass_kernel_spmd

B, S, D = 8, 2048, 2048
N = 2048                 # S == D
KT = N // 128            # 16 contraction tiles
NCHUNK = N // 512        # 4 free-dim chunks of 512
F16 = mybir.dt.float16
F32 = mybir.dt.float32
AX = mybir.AxisListType.X
EXP = mybir.ActivationFunctionType.Exp

_compiled = {}


def _build():
    nc = bacc.Bacc("TRN2", target_bir_lowering=False, debug=False)

    # ExternalInputs (per core). x* are host-transposed activations [d, s] fp16.
    xq = nc.dram_tensor("xq", [N, N], F16, kind="ExternalInput").ap()
    xk = nc.dram_tensor("xk", [N, N], F16, kind="ExternalInput").ap()
    xv = nc.dram_tensor("xv", [N, N], F16, kind="ExternalInput").ap()
    # host-transposed weights [d, e] fp16
    wq = nc.dram_tensor("wq", [N, N], F16, kind="ExternalInput").ap()
    wk = nc.dram_tensor("wk", [N, N], F16, kind="ExternalInput").ap()
    wv = nc.dram_tensor("wv", [N, N], F16, kind="ExternalInput").ap()
    # biases: bqb/bvb broadcast across partitions [128, N]; bkp partition-major [128, 16]
    bqb = nc.dram_tensor("bqb", [128, N], F32, kind="ExternalInput").ap()
    bkp = nc.dram_tensor("bkp", [128, KT], F32, kind="ExternalInput").ap()
    bvb = nc.dram_tensor("bvb", [128, N], F32, kind="ExternalInput").ap()

    out = nc.dram_tensor("out", [N, N], F16, kind="ExternalOutput").ap()

    with tile.TileContext(nc, pool_alloc_mode="queue") as tc:
        with tc.tile_pool(name="dram", bufs=1, space="DRAM") as dram:
            q_f = dram.tile([N, N], F16, tag="q_f")
            k_f = dram.tile([N, N], F16, tag="k_f")
            v_f = dram.tile([N, N], F16, tag="v_f")
            at_f = dram.tile([N, N], F16, tag="at_f")

            with tc.tile_pool(name="psum", bufs=8, space="PSUM") as psum:
                _proj_rows(nc, tc, psum, xq, wq, bqb, q_f, "q", cold=True)
                _proj_cols(nc, tc, psum, xk, wk, bkp, k_f)
                _proj_rows(nc, tc, psum, xv, wv, bvb, v_f, "v")
                _attention(nc, tc, psum, q_f, k_f, v_f, at_f, out)

    nc.compile()
    return nc


def _load_kblock(nc, pool, dram_ap, col_blk, tag, dt=F16, parts=1):
    """Load DRAM[:, col_blk*128 : +128] ([N, 128]) into one [128, N] SBUF tile
    whose slice [:, k*128:(k+1)*128] is contraction-tile k (partition = row%128).
    parts>1 splits the transfer across that many DMA queues (cold start)."""
    t = pool.tile([128, N], dt, tag=tag)
    seg = N // parts
    for h in range(parts):
        src = dram_ap[h * seg:(h + 1) * seg,
                      col_blk * 128:(col_blk + 1) * 128].rearrange(
            "(t p) s -> p t s", p=128
        )
        dst = t[:, h * seg:(h + 1) * seg].rearrange(
            "p (t s) -> p t s", t=KT // parts)
        nc.sync.dma_start(dst, src)
    return t


def _load_resident(nc, pool, src, tag, split0=False, defer=False):
    """Load an [N, N] fp16 DRAM tensor as KT resident [128, N] row-block tiles,
    chunk-0 columns first, on the SWDGE path (separate from HWDGE streaming).
    split0 puts half of chunk 0 on the sync path to halve cold-start latency.
    defer returns (tiles, emit) so the caller controls per-chunk emission order."""
    ts = [pool.tile([128, N], F16, tag=f"{tag}{k}", name=f"{tag}{k}") for k in range(KT)]

    def emit(chunks):
        for c in chunks:
            cs = slice(c * 512, (c + 1) * 512)
            for k in range(KT):
                eng = nc.sync if (split0 and c == 0 and k % 2 == 1) else nc.gpsimd
                eng.dma_start(ts[k][:, cs], src[k * 128:(k + 1) * 128, cs])

    if defer:
        return ts, emit
    emit(range(NCHUNK))
    return ts


def _proj_rows(nc, tc, psum, x, w, bias_bcast, out_f, pfx, cold=False):
    """Row-major projection: out[s, e] = sum_d X^T[d, s] * W^T[d, e] + bias[e].
    Stationary = activation k-blocks, moving = resident weights.
    cold: run the first two row-tiles chunk-major so the per-chunk demand
    on the streaming weight loads is halved while HBM ramps up."""
    with (
        tc.tile_pool(name=f"p{pfx}_w", bufs=1) as wpool,
        tc.tile_pool(name=f"p{pfx}_x", bufs=6) as xpool,
        tc.tile_pool(name=f"p{pfx}_s", bufs=4) as spool,
        tc.tile_pool(name=f"p{pfx}_b", bufs=1) as bpool,
    ):
        if cold:
            # a0 first, then weight chunk 0 on BOTH dma paths (the PE sweeps
            # all 16 k-tiles of chunk 0 within ~4us of the first matmul),
            # then the remaining a-blocks, bias, and weight chunks 1-3
            pend = {0: _load_kblock(nc, xpool, x, 0, f"{pfx}a", parts=4)}
            w_t, emit_w = _load_resident(nc, wpool, w, f"{pfx}w",
                                         split0=True, defer=True)
            emit_w([0])
            for s, p in ((1, 2), (2, 2), (3, 1)):
                pend[s] = _load_kblock(nc, xpool, x, s, f"{pfx}a", parts=p)
            bb = bpool.tile([128, N], F32, tag="bias")
            nc.sync.dma_start(bb[:], bias_bcast[:])
            emit_w(range(1, NCHUNK))
        else:
            pend = {0: _load_kblock(nc, xpool, x, 0, f"{pfx}a")}
            bb = bpool.tile([128, N], F32, tag="bias")
            nc.sync.dma_start(bb[:], bias_bcast[:])
            w_t = _load_resident(nc, wpool, w, f"{pfx}w", split0=True)

        def row_chunk(a, s, c):
            cs = slice(c * 512, (c + 1) * 512)
            ps = psum.tile([128, 512], F32)
            for k in range(KT):
                nc.tensor.matmul(ps[:], a[:, k * 128:(k + 1) * 128],
                                 w_t[k][:, cs], start=(k == 0), stop=(k == KT - 1))
            o16 = spool.tile([128, 512], F16, tag="o16")
            nc.vector.tensor_add(o16[:], ps[:], bb[:, cs])
            nc.sync.dma_start(out_f[s * 128:(s + 1) * 128, cs], o16[:])

        start_s = 0
        if cold:
            colds = [pend.pop(s) for s in range(4)]
            pend[4] = _load_kblock(nc, xpool, x, 4, f"{pfx}a")
            for c in range(NCHUNK):
                for s, a in enumerate(colds):
                    row_chunk(a, s, c)
            start_s = 4
        for s in range(start_s, KT):
            if s + 1 < KT:
                pend[s + 1] = _load_kblock(nc, xpool, x, s + 1, f"{pfx}a")
            a = pend.pop(s)
            for c in range(NCHUNK):
                row_chunk(a, s, c)


def _proj_cols(nc, tc, psum, x, w, bias_part, out_f):
    """kT-style projection: out[e, s] = sum_d W^T[d, e] * X^T[d, s] + bias[e].
    Stationary = weight k-blocks, moving = resident activations."""
    with (
        tc.tile_pool(name="pc_x", bufs=1) as xpool,
        tc.tile_pool(name="pc_w", bufs=3) as wpool,
        tc.tile_pool(name="pc_s", bufs=4) as spool,
        tc.tile_pool(name="pc_b", bufs=1) as bpool,
    ):
        pend = {0: _load_kblock(nc, wpool, w, 0, "kg")}
        bp = bpool.tile([128, KT], F32, tag="biasp")
        nc.sync.dma_start(bp[:], bias_part[:])
        x_t = _load_resident(nc, xpool, x, "kx")
        for e in range(KT):
            if e + 1 < KT:
                pend[e + 1] = _load_kblock(nc, wpool, w, e + 1, "kg")
            g = pend.pop(e)
            for c in range(NCHUNK):
                cs = slice(c * 512, (c + 1) * 512)
                ps = psum.tile([128, 512], F32)
                for k in range(KT):
                    nc.tensor.matmul(ps[:], g[:, k * 128:(k + 1) * 128],
                                     x_t[k][:, cs], start=(k == 0), stop=(k == KT - 1))
                o16 = spool.tile([128, 512], F16, tag="o16")
                nc.vector.tensor_scalar_add(o16[:], ps[:], bp[:, e:e + 1])
                nc.sync.dma_start(out_f[e * 128:(e + 1) * 128, cs], o16[:])


LAG = 6                   # row-tiles attn@v trails behind scores
GRP = 4                   # row-tiles per transpose group


def _attention(nc, tc, psum, q_f, k_f, v_f, at_f, out):
    """Interleaved scores+softmax+attn@v, one 128-row attn block at a time.

    scores[i, j] = sum_m q[m, i]*kT[m, j]; row softmax -> a16 -> at_f DRAM.
    Every GRP row-tiles, 16 [GRP*128, 128] DRAM->SBUF transposes (scalar
    HWDGE) build attn^T group tiles (j on partitions). attn@v for row-tile
    i runs LAG row-tiles behind: ov[i, n] = sum_j attn[i, j] * v[j, n],
    with resident v row-blocks as the moving operand. out is written in
    [i, n] orientation; the host transposes.
    """
    with (
        tc.tile_pool(name="sc_k", bufs=1) as kpool,
        tc.tile_pool(name="av_v", bufs=1) as vpool,
        tc.tile_pool(name="sc_a", bufs=2) as apool,
        tc.tile_pool(name="sc_q", bufs=3) as qpool,
        tc.tile_pool(name="sc_c", bufs=2) as cpool,
        tc.tile_pool(name="sc_s", bufs=2) as spool,
        tc.tile_pool(name="sc_t", bufs=4) as tpool,
        tc.tile_pool(name="sc_r", bufs=8) as rpool,
        tc.tile_pool(name="av_s", bufs=4) as opool,
    ):
        k_t = _load_resident(nc, kpool, k_f, "sk")
        v_r = _load_resident(nc, vpool, v_f, "vr")
        attg_of = {}
        rcp_of = {}

        def emit_av(i):
            attg = attg_of[i // GRP]
            ts = slice((i % GRP) * 128, (i % GRP) * 128 + 128)
            rcp = rcp_of.pop(i)
            for c in range(NCHUNK):
                cs = slice(c * 512, (c + 1) * 512)
                ps = psum.tile([128, 512], F32)
                for j in range(KT):
                    nc.tensor.matmul(ps[:], attg[j][:, ts],
                                     v_r[j][:, cs], start=(j == 0), stop=(j == KT - 1))
                # attn rows are stored unnormalized; fold the softmax
                # 1/rowsum into the output copy (per-partition scalar)
                o16 = opool.tile([128, 512], F16, tag="o16")
                nc.vector.tensor_scalar_mul(o16[:], ps[:], rcp[:])
                nc.sync.dma_start(out[i * 128:(i + 1) * 128, cs], o16[:])

        qpend = {0: _load_kblock(nc, qpool, q_f, 0, "sq"),
                 1: _load_kblock(nc, qpool, q_f, 1, "sq")}
        for i in range(KT):
            if i + 2 < KT:
                qpend[i + 2] = _load_kblock(nc, qpool, q_f, i + 2, "sq")
            qh = qpend.pop(i)
            m4 = tpool.tile([128, NCHUNK], F32, tag="m4")
            sc32 = cpool.tile([128, N], F32, tag="sc32")
            for c in range(NCHUNK):
                cs = slice(c * 512, (c + 1) * 512)
                ps = psum.tile([128, 512], F32)
                for k in range(KT):
                    nc.tensor.matmul(ps[:], qh[:, k * 128:(k + 1) * 128],
                                     k_t[k][:, cs], start=(k == 0), stop=(k == KT - 1))
                nc.vector.reduce_max(m4[:, c:c + 1], ps[:], axis=AX)
                # drain the PSUM bank immediately so scores never starve
                # on banks even when the scalar engine runs behind
                nc.vector.tensor_copy(sc32[:, cs], ps[:])
            mx = tpool.tile([128, 1], F32, tag="mx")
            nc.vector.reduce_max(mx[:], m4[:], axis=AX)
            negm = tpool.tile([128, 1], F32, tag="negm")
            nc.scalar.mul(negm[:], mx[:], -1.0)
            a16 = spool.tile([128, N], F16, tag="a16")
            sume = tpool.tile([128, NCHUNK], F32, tag="sume")
            for c in range(NCHUNK):
                cs = slice(c * 512, (c + 1) * 512)
                nc.scalar.activation(a16[:, cs], sc32[:, cs], EXP,
                                     bias=negm[:], scale=1.0,
                                     accum_out=sume[:, c:c + 1])
            tot = tpool.tile([128, 1], F32, tag="tot")
            nc.vector.reduce_sum(tot[:], sume[:], axis=AX)
            rcp = rpool.tile([128, 1], F32, tag="rcp")
            nc.vector.reciprocal(rcp[:], tot[:])
            rcp_of[i] = rcp
            nc.sync.dma_start(at_f[i * 128:(i + 1) * 128, :], a16[:])
            if i % GRP == GRP - 1:
                g = i // GRP
                attg = [apool.tile([128, GRP * 128], F16, tag=f"ag{j}",
                                   name=f"ag{j}") for j in range(KT)]
                for j in range(KT):
                    nc.scalar.dma_start_transpose(
                        attg[j][:],
                        at_f[g * GRP * 128:(g + 1) * GRP * 128,
                             j * 128:(j + 1) * 128])
                attg_of[g] = attg
            if i >= LAG:
                emit_av(i - LAG)
        for i in range(KT - LAG, KT):
            emit_av(i)


def prepare_in_maps(query, key_, value, Wq, bq, Wk, bk, Wv, bv):
    query = np.asarray(query, dtype=np.float32)
    key_ = np.asarray(key_, dtype=np.float32)
    value = np.asarray(value, dtype=np.float32)
    Wq = np.asarray(Wq, dtype=np.float32)
    Wk = np.asarray(Wk, dtype=np.float32)
    Wv = np.asarray(Wv, dtype=np.float32)
    bq = np.asarray(bq, dtype=np.float32)
    bk = np.asarray(bk, dtype=np.float32)
    bv = np.asarray(bv, dtype=np.float32)

    wqt = np.ascontiguousarray(Wq.T).astype(np.float16)
    wkt = np.ascontiguousarray(Wk.T).astype(np.float16)
    wvt = np.ascontiguousarray(Wv.T).astype(np.float16)
    bqb = np.broadcast_to(bq, (128, N)).copy()
    bvb = np.broadcast_to(bv, (128, N)).copy()
    bkp = np.ascontiguousarray(bk.reshape(KT, 128).T)

    in_maps = []
    for b in range(B):
        in_maps.append({
            "xq": np.ascontiguousarray(query[b].T).astype(np.float16),
            "xk": np.ascontiguousarray(key_[b].T).astype(np.float16),
            "xv": np.ascontiguousarray(value[b].T).astype(np.float16),
            "wq": wqt, "wk": wkt, "wv": wvt,
            "bqb": bqb, "bkp": bkp, "bvb": bvb,
        })
    return in_maps


def get_nc():
    if "nc" not in _compiled:
        _compiled["nc"] = _build()
    return _compiled["nc"]


def kernel(query, key_, value, Wq, bq, Wk, bk, Wv, bv):
    in_maps = prepare_in_maps(query, key_, value, Wq, bq, Wk, bk, Wv, bv)
    res = run_bass_kernel_spmd(get_nc(), in_maps, core_ids=list(range(B)))
    # device emits (attn @ v)[i, n]; the module's output is its transpose
    return np.stack([np.asarray(res.results[b]["out"]).T for b in range(B)]).astype(np.float32)


if __name__ == "__main__":
    rng = np.random.default_rng(0)
    inputs = {
        "query": rng.standard_normal((B, S, D), dtype=np.float32),
        "key_": rng.standard_normal((B, S, D), dtype=np.float32),
        "value": rng.standard_normal((B, S, D), dtype=np.float32),
        "Wq": (rng.standard_normal((D, D), dtype=np.float32) / np.sqrt(D)),
        "bq": rng.standard_normal(D).astype(np.float32) * 0.01,
        "Wk": (rng.standard_normal((D, D), dtype=np.float32) / np.sqrt(D)),
        "bk": rng.standard_normal(D).astype(np.float32) * 0.01,
        "Wv": (rng.standard_normal((D, D), dtype=np.float32) / np.sqrt(D)),
        "bv": rng.standard_normal(D).astype(np.float32) * 0.01,
    }
    out = kernel(**inputs)
    print("out", out.shape, out.dtype)
